# revision 1
# baseline (speedup 1.0000x reference)
"""Trainium2 Bass kernel for top-2 MoE (nn_ExpertMemory).

Model (reference semantics):
    logits = x @ gate_w + gate_b          # (N, E)
    probs  = softmax(logits)
    gates, idx = top_k(probs, 2)
    out[n] = sum_k gates[n,k] * (relu(x[n] @ w1[e] + b1[e]) @ w2[e] + b2[e]),
             e = idx[n,k]
(The reference runs every expert densely, but combine weights are zero off
the top-2, so routed computation is mathematically identical.)

Strategy: data-parallel over tokens across 8 NeuronCores (1024 tokens each).
Each core, fully on device:
  1. gate matmul (true fp32) + softmax + top-2 on its tokens; per-(token,
     rank) gate values are also written to a small DRAM table ginter[2t+r+1]
  2. per-expert token lists via sparse_gather over candidate encodings;
     the junk tail beyond the found count (HW leaves arbitrary values) is
     neutralized with an on-device count mask
  3. dispatch via dma_gather(transpose=True) of token rows from DRAM
     directly into C-major SBUF layout; slot-ordered gate values are
     fetched with a second (tiny) dma_gather from ginter
  4. layer 1 slot-moving (out [h, slots], b1 as activation bias); layer 2
     with h as the stationary operand so the output lands token-major
     [slots, C] in PSUM; the gate is applied as the per-partition `scale`
     of the PSUM->SBUF copy
  5. combine via dma_scatter_add of bf16 rows into yT, which is
     pre-initialized with the gate-weighted b2 correction
     (sum_r g_r*b2[e_r]) computed by a tiny matmul. Pad slots scatter to a
     trash row: CCE read-modify-write is not atomic, so they must never
     alias real rows of the same scatter.
All tile pools live outside the repeat loop and the routing staging buffers
(cbuf/ginter) are double-buffered by repeat parity, so consecutive
iterations pipeline (stage A of rep i+1 overlaps stage B of rep i).
Slot capacities are specialized per run from a host-side replica of the
routing (inputs only, margin 16); the device computes everything itself.
"""

import numpy as np
from contextlib import ExitStack

import concourse.bass as bass
import concourse.tile as tile
import concourse.mybir as mybir
from concourse import bacc

dt = mybir.dt
AF = mybir.ActivationFunctionType
ALU = mybir.AluOpType
AX = mybir.AxisListType

P = 128

# problem constants
B, T, C, E, H, TOPK = 4, 2048, 1024, 8, 2048, 2
NCORE = 8
NTOK = B * T // NCORE  # tokens per core
TCH = NTOK // P        # token chunks (8)
CK = C // P            # C chunks (8)
HK = H // P            # H chunks (16)
GW64 = 64              # ginter row width (64 f32 = 256 B, gather minimum)
NH = NTOK // 512       # 512-wide token halves for the gate matmul


def _tiles(s):
    """Split slot range s into moving tiles: full 512s, then the remainder
    (16-aligned). Tile starts are 128-aligned so L2 sub-tiles line up with
    the global slot chunks."""
    out = []
    off = 0
    rem = s
    while rem > 512:
        out.append((off, 512))
        off += 512
        rem -= 512
    if rem:
        out.append((off, rem))
    return out


def build_program(S, ntok=NTOK, level=9, repeat=1):
    nc = _build(S, ntok=ntok, level=level, repeat=repeat)
    nc.compile()
    return nc


def _build(S, ntok=NTOK, level=9, repeat=1):
    """S: per-expert slot capacities (multiples of 16, each <= 512)."""
    S = [int(s) for s in S]
    assert all(s % 16 == 0 and 16 <= s <= 512 for s in S)
    S128 = [(s + 127) // 128 * 128 for s in S]

    nc = bacc.Bacc("TRN2", target_bir_lowering=False, debug=False,
                   num_swdge_queues=2)

    f32, bf16 = dt.float32, dt.bfloat16
    xT = nc.dram_tensor("xT", [C, ntok], f32, kind="ExternalInput").ap()
    xtm = nc.dram_tensor("xtm", [ntok, C], bf16, kind="ExternalInput").ap()
    gw = nc.dram_tensor("gw", [C, E], f32, kind="ExternalInput").ap()
    gb = nc.dram_tensor("gb", [E, 1], f32, kind="ExternalInput").ap()
    w1 = nc.dram_tensor("w1", [E, HK, P, CK * P], bf16,
                        kind="ExternalInput").ap()
    b1 = nc.dram_tensor("b1", [E, H, 1], f32, kind="ExternalInput").ap()
    w2 = nc.dram_tensor("w2", [E, H, C], bf16, kind="ExternalInput").ap()
    b2e = nc.dram_tensor("b2e", [E, C], f32, kind="ExternalInput").ap()
    id8 = nc.dram_tensor("id8", [E, E], f32, kind="ExternalInput").ap()
    id128 = nc.dram_tensor("id128", [P, P], f32, kind="ExternalInput").ap()
    # +16 rows: trash target for pad-slot scatter writes (their payload is
    # zero, but pointing them at real rows would race the real adds within
    # the same scatter DMA — CCE read-modify-write is not atomic)
    yT = nc.dram_tensor("yT", [ntok + 16, C], bf16, kind="ExternalOutput").ap()

    # routing staging, double-buffered by repeat parity
    cbufG = nc.dram_tensor("cbufG", [2, E, ntok], f32).ap()  # 2t+r+1 | -1
    cbufT = nc.dram_tensor("cbufT", [2, E, ntok], f32).ap()  # t+1 | -1
    # per-(token, rank) gate values, row 1+2t+r; row 0 zeroed for pads
    ginter = nc.dram_tensor("ginter", [2, 2 * ntok + 16, GW64], f32).ap()

    with tile.TileContext(nc) as tc, ExitStack() as ctx:
        cpool = ctx.enter_context(tc.tile_pool(name="const", bufs=1))
        gpool = ctx.enter_context(tc.tile_pool(name="gk", bufs=2))
        sa = ctx.enter_context(tc.tile_pool(name="sa", bufs=2))
        sa1 = ctx.enter_context(tc.tile_pool(name="sa1", bufs=2))
        xtp = ctx.enter_context(tc.tile_pool(name="xt", bufs=3))
        mb = ctx.enter_context(tc.tile_pool(name="mb", bufs=3))
        w1p = ctx.enter_context(tc.tile_pool(name="w1p", bufs=6))
        w2p = ctx.enter_context(tc.tile_pool(name="w2p", bufs=2))
        xgp = ctx.enter_context(tc.tile_pool(name="xgp", bufs=3))
        hp = ctx.enter_context(tc.tile_pool(name="hp", bufs=2))
        ystp = ctx.enter_context(tc.tile_pool(name="ystp", bufs=2))
        ycp = ctx.enter_context(tc.tile_pool(name="ycp", bufs=1))
        pgp = ctx.enter_context(tc.tile_pool(name="pgp", bufs=1,
                                             space="PSUM"))
        pms = ctx.enter_context(tc.tile_pool(name="pms", bufs=1,
                                             space="PSUM"))
        p1 = ctx.enter_context(tc.tile_pool(name="p1", bufs=2, space="PSUM"))
        p2 = ctx.enter_context(tc.tile_pool(name="p2", bufs=2, space="PSUM"))

        # ---- constants (loaded once) ----
        gwsb = cpool.tile([P, CK * E], f32)
        nc.sync.dma_start(gwsb[:].rearrange("p (k e) -> p k e", e=E),
                          gw.rearrange("(k p) e -> p k e", p=P))
        id8sb = cpool.tile([E, E], f32)
        nc.sync.dma_start(id8sb[:], id8)
        id128sb = cpool.tile([P, P], f32)
        nc.sync.dma_start(id128sb[:], id128)
        gbsb = cpool.tile([E, 1], f32)
        nc.sync.dma_start(gbsb[:], gb)
        b2sb = cpool.tile([E, C], f32)
        nc.sync.dma_start(b2sb[:], b2e)
        iotaE_i = cpool.tile([P, TCH * E], dt.int32)
        nc.gpsimd.iota(iotaE_i[:], pattern=[[0, TCH], [1, E]], base=0,
                       channel_multiplier=0)
        iotaE = cpool.tile([P, TCH * E], f32)
        nc.vector.tensor_copy(iotaE[:], iotaE_i[:])
        toks_i = cpool.tile([P, TCH], dt.int32)
        nc.gpsimd.iota(toks_i[:], pattern=[[P, TCH]], base=0,
                       channel_multiplier=1)
        toksf = cpool.tile([P, TCH], f32)
        nc.vector.tensor_copy(toksf[:], toks_i[:])
        slotio_i = cpool.tile([16, 512 // 16], dt.int32)
        nc.gpsimd.iota(slotio_i[:], pattern=[[16, 512 // 16]], base=0,
                       channel_multiplier=1)
        slotio = cpool.tile([16, 512 // 16], f32)
        nc.vector.tensor_copy(slotio[:], slotio_i[:])
        ones16 = cpool.tile([P, 16], f32)
        nc.vector.memset(ones16[:], 1.0)

        def stage_a(rep):
            par = rep % 2
            cbG = cbufG[par]
            cbT = cbufT[par]
            gint = ginter[par]

            # =============== Stage A: gate + routing ===============
            # gate logits, expert-major: lgT[e, tok]. True fp32 matmul:
            # fp32r is reduced-precision on HW and would flip top-2 picks.
            lgT = sa1.tile([E, ntok], f32, tag="lgT")
            for nh in range(NH):
                lgps = pgp.tile([E, 512], f32, space="PSUM", tag="lgp")
                for k in range(CK):
                    xt = xtp.tile([P, 512], f32, tag="xt")
                    nc.sync.dma_start(
                        xt[:], xT[k * P:(k + 1) * P,
                                  nh * 512:(nh + 1) * 512])
                    nc.tensor.matmul(lgps[:],
                                     lhsT=gwsb[:, k * E:(k + 1) * E],
                                     rhs=xt[:],
                                     start=(k == 0), stop=(k == CK - 1))
                nc.vector.tensor_scalar_add(lgT[:, nh * 512:(nh + 1) * 512],
                                            lgps[:], gbsb[:, :1])
            # transpose to token-major [128, TCH, e]
            lg = sa1.tile([P, TCH, E], f32, tag="lg")
            for t in range(TCH):
                ps = pms.tile([P, E], f32, space="PSUM", tag="misc")
                nc.tensor.transpose(ps[:], lgT[:, t * P:(t + 1) * P],
                                    id8sb[:])
                nc.vector.tensor_copy(lg[:, t, :], ps[:])
            # softmax over experts
            mx = sa.tile([P, TCH], f32, tag="mx")
            nc.vector.tensor_reduce(mx[:], lg[:], axis=AX.X, op=ALU.max)
            xm = sa.tile([P, TCH, E], f32, tag="xm")
            nc.vector.tensor_tensor(out=xm[:], in0=lg[:],
                                    in1=mx[:].to_broadcast([P, TCH, E]),
                                    op=ALU.subtract)
            ex = sa.tile([P, TCH, E], f32, tag="ex")
            nc.scalar.activation(ex[:], xm[:], AF.Exp)
            sm = sa.tile([P, TCH], f32, tag="sm")
            nc.vector.tensor_reduce(sm[:], ex[:], axis=AX.X, op=ALU.add)
            rs = sa.tile([P, TCH], f32, tag="rs")
            nc.vector.reciprocal(rs[:], sm[:])
            probs = sa.tile([P, TCH, E], f32, tag="probs")
            nc.vector.tensor_tensor(out=probs[:], in0=ex[:],
                                    in1=rs[:].to_broadcast([P, TCH, E]),
                                    op=ALU.mult)
            # top-2 by logits (same order as by probs)
            mig = sa.tile([P, TCH, 8], dt.uint32, tag="mig")
            for t in range(TCH):
                mv = sa.tile([P, 8], f32, tag="mv")
                nc.vector.max(mv[:], lg[:, t, :])
                nc.vector.max_index(mig[:, t, :], mv[:], lg[:, t, :])
            migf = sa.tile([P, TCH, 8], f32, tag="migf")
            nc.vector.tensor_copy(migf[:], mig[:])

            A = []  # one-hot masks per rank [P, TCH, e]
            g = []
            for r in range(2):
                Ar = sa1.tile([P, TCH, E], f32, tag=f"A{r}")
                nc.vector.tensor_tensor(
                    out=Ar[:],
                    in0=migf[:, :, r:r + 1].to_broadcast([P, TCH, E]),
                    in1=iotaE[:].rearrange("p (t e) -> p t e", e=E),
                    op=ALU.is_equal)
                gr = gpool.tile([P, TCH], f32, tag=f"g{r}")
                tmp = sa.tile([P, TCH, E], f32, tag="gt")
                nc.vector.tensor_tensor(out=tmp[:], in0=probs[:], in1=Ar[:],
                                        op=ALU.mult)
                nc.vector.tensor_reduce(gr[:], tmp[:], axis=AX.X, op=ALU.add)
                A.append(Ar)
                g.append(gr)
            M = sa1.tile([P, TCH, E], f32, tag="M")
            nc.vector.tensor_tensor(out=M[:], in0=A[0][:], in1=A[1][:],
                                    op=ALU.add)

            # per-expert token counts, replicated on 16 partitions (used to
            # mask off sparse_gather's junk tail beyond the found count)
            Mre = sa.tile([P, E, TCH], f32, tag="Mre")
            nc.vector.tensor_copy(Mre[:], M[:].rearrange("p t e -> p e t"))
            cntp = pms.tile([16, E * TCH], f32, space="PSUM", tag="misc")
            nc.tensor.matmul(cntp[:], lhsT=ones16[:],
                             rhs=Mre[:].rearrange("p e t -> p (e t)"),
                             start=True, stop=True)
            cntet = sa.tile([16, E, TCH], f32, tag="cntet")
            nc.vector.tensor_copy(cntet[:],
                                  cntp[:].rearrange("p (e t) -> p e t", e=E))
            cnt16 = gpool.tile([16, E], f32, tag="cnt16")
            nc.vector.tensor_reduce(cnt16[:], cntet[:], axis=AX.X, op=ALU.add)

            if level < 1:
                return None
            # candidate encodings (+1-shifted so sparse-gather pads, which
            # are <= 0, can be clamped to the zero row / token 0):
            #   G = 2*tok + r + 1 (else -1), T = tok + 1 (else -1)
            tokp2 = sa.tile([P, TCH], f32, tag="tokp2")
            nc.vector.tensor_scalar_add(tokp2[:], toksf[:], 2.0)
            tok2 = sa.tile([P, TCH], f32, tag="tok2")
            nc.vector.tensor_scalar(tok2[:], toksf[:], 2.0, 2.0,
                                    op0=ALU.mult, op1=ALU.add)
            candG = sa1.tile([P, TCH, E], f32, tag="candG")
            nc.vector.tensor_tensor(
                out=candG[:], in0=tok2[:].to_broadcast([P, TCH, E]),
                in1=M[:], op=ALU.mult)
            nc.vector.tensor_tensor(out=candG[:], in0=candG[:], in1=A[1][:],
                                    op=ALU.add)
            nc.vector.tensor_scalar_add(candG[:], candG[:], -1.0)
            candT = sa1.tile([P, TCH, E], f32, tag="candT")
            nc.vector.tensor_tensor(
                out=candT[:], in0=tokp2[:].to_broadcast([P, TCH, E]),
                in1=M[:], op=ALU.mult)
            nc.vector.tensor_scalar_add(candT[:], candT[:], -1.0)
            for ei in range(E):
                nc.scalar.dma_start(
                    cbG[ei, :].rearrange("(t p) -> p t", p=P),
                    candG[:, :, ei])
                nc.scalar.dma_start(
                    cbT[ei, :].rearrange("(t p) -> p t", p=P),
                    candT[:, :, ei])

            # ---- ginter: per-(token, rank) gates, rows 1 + 2t + r ----
            zrow = sa.tile([1, GW64], f32, tag="zrow")
            nc.vector.memset(zrow[:], 0.0)
            nc.scalar.dma_start(gint[0:1, :], zrow[:])
            for r in range(2):
                g64 = sa.tile([P, TCH, GW64], f32, tag=f"g64_{r}")
                nc.vector.tensor_scalar_add(
                    g64[:], g[r][:].to_broadcast([P, TCH, GW64]), 0.0)
                nc.scalar.dma_start(
                    gint[1:1 + 2 * ntok, :].rearrange(
                        "(tch p two) f -> p tch two f",
                        p=P, two=2)[:, :, r, :],
                    g64[:])

            # ---- yT init: sum_r g_r * b2[e_r] ----
            wtok = sa1.tile([P, TCH, E], f32, tag="wtok")
            nc.vector.tensor_tensor(
                out=wtok[:], in0=A[0][:],
                in1=g[0][:].to_broadcast([P, TCH, E]), op=ALU.mult)
            wtk1 = sa.tile([P, TCH, E], f32, tag="wtk1")
            nc.vector.tensor_tensor(
                out=wtk1[:], in0=A[1][:],
                in1=g[1][:].to_broadcast([P, TCH, E]), op=ALU.mult)
            nc.vector.tensor_tensor(out=wtok[:], in0=wtok[:], in1=wtk1[:],
                                    op=ALU.add)
            wTe = sa1.tile([E, TCH * P], f32, tag="wTe")
            for t in range(TCH):
                pw = pms.tile([E, P], f32, space="PSUM", tag="misc")
                nc.tensor.transpose(pw[:], wtok[:, t, :], id128sb[:])
                nc.vector.tensor_copy(wTe[:, t * P:(t + 1) * P], pw[:])
            ycorr = ycp.tile([P, TCH, C], bf16, tag="ycorr")
            for t in range(TCH):
                for hh in range(2):
                    pc = pms.tile([P, 512], f32, space="PSUM", tag="misc")
                    nc.tensor.matmul(pc[:], lhsT=wTe[:, t * P:(t + 1) * P],
                                     rhs=b2sb[:, hh * 512:(hh + 1) * 512],
                                     start=True, stop=True)
                    nc.vector.tensor_copy(
                        ycorr[:, t, hh * 512:(hh + 1) * 512], pc[:])
            return dict(cnt16=cnt16, ycorr=ycorr, cbG=cbG, cbT=cbT,
                        gint=gint)

        def stage_b(rep, actx):
            cnt16 = actx["cnt16"]
            cbG, cbT, gint = actx["cbG"], actx["cbT"], actx["gint"]
            # yT init (emitted here so the WAW chain with the previous
            # rep's scatter-adds stays in the right order)
            nc.sync.dma_start(
                yT[0:ntok, :].rearrange("(tch p) c -> p tch c", p=P),
                actx["ycorr"][:])
            for ei in range(E):
                Se = S[ei]
                Sg = S128[ei]
                nsub = (Se + 127) // 128
                w16 = Sg // 16
                # ---- token lists ----
                cwG = mb.tile([16, ntok // 16], f32, tag="cwG")
                nc.scalar.dma_start(
                    cwG[:], cbG[ei, :].rearrange("(f p) -> p f", p=16))
                cwT = mb.tile([16, ntok // 16], f32, tag="cwT")
                nc.scalar.dma_start(
                    cwT[:], cbT[ei, :].rearrange("(f p) -> p f", p=16))

                # junk-tail mask: slots >= count are diverted/neutralized
                msk = mb.tile([16, w16], f32, tag="msk")
                nc.vector.tensor_tensor(
                    out=msk[:], in0=slotio[:, :w16],
                    in1=cnt16[:, ei:ei + 1].to_broadcast([16, w16]),
                    op=ALU.is_lt)

                tkT = mb.tile([16, w16], f32, tag="tkT")
                nc.vector.memset(tkT[:], -1.0)
                nfdT = mb.tile([1, 1], dt.uint32, tag="nfdT")
                nc.gpsimd.sparse_gather(tkT[:, :Se // 16], cwT[:],
                                        num_found=nfdT[:])
                tkG = mb.tile([16, w16], f32, tag="tkG")
                nc.vector.memset(tkG[:], -1.0)
                nfdG = mb.tile([1, 1], dt.uint32, tag="nfdG")
                nc.gpsimd.sparse_gather(tkG[:, :Se // 16], cwG[:],
                                        num_found=nfdG[:])

                # x-row list: token ids, pads/junk clamped into range
                xff = mb.tile([16, w16], f32, tag="xff")
                nc.vector.tensor_scalar(xff[:], tkT[:], 1.0, float(ntok),
                                        op0=ALU.max, op1=ALU.min)
                nc.vector.tensor_scalar_add(xff[:], xff[:], -1.0)
                # scatter list: same tokens, junk tail -> trash row
                stf = mb.tile([16, Se // 16], f32, tag="stf")
                nc.vector.scalar_tensor_tensor(
                    out=stf[:], in0=xff[:, :Se // 16], scalar=float(-ntok),
                    in1=msk[:, :Se // 16], op0=ALU.add, op1=ALU.mult)
                nc.vector.tensor_scalar_add(stf[:], stf[:], float(ntok))
                # gate list: rows 1+2t+r of ginter, pads -> zero row
                gvf = mb.tile([16, w16], f32, tag="gvf")
                nc.vector.tensor_scalar(gvf[:], tkG[:], 0.0, float(2 * ntok),
                                        op0=ALU.max, op1=ALU.min)

                # single replicated index tile: [X | T | G]
                trip = mb.tile([P, 3 * w16], dt.int16, tag="trip")
                nc.vector.tensor_copy(trip[0:16, 0:w16], xff[:])
                nc.vector.tensor_copy(trip[0:16, w16:w16 + Se // 16], stf[:])
                nc.vector.tensor_copy(trip[0:16, 2 * w16:3 * w16], gvf[:])
                for sz in (16, 32, 64):
                    nc.scalar.dma_start(trip[sz:2 * sz, :], trip[0:sz, :])
                t16X = trip[:, 0:w16]
                t16T = trip[:, w16:w16 + Se // 16]
                t16G = trip[:, 2 * w16:3 * w16]

                # ---- dispatch gathers ----
                xg = xgp.tile([P, CK, Sg], bf16, tag="xg")
                nc.gpsimd.dma_gather(xg[:], xtm, t16X, Sg, Sg, C,
                                     transpose=True)
                ggt = xgp.tile([P, Sg // 128, GW64], f32, tag="gg")
                nc.gpsimd.dma_gather(ggt[:], gint, t16G, Sg, Sg, GW64)

                b1e = mb.tile([P, HK], f32, tag="b1e")
                nc.scalar.dma_start(
                    b1e[:].rearrange("p (k o) -> p k o", o=1),
                    b1[ei].rearrange("(k p) one -> p k one", p=P))

                # ---- w2 resident for this expert ----
                w2t = []
                for hk in range(HK):
                    wt = w2p.tile([P, C], bf16, tag=f"w2_{hk}")
                    nc.sync.dma_start(
                        wt[:], w2[ei, hk * P:(hk + 1) * P, :])
                    w2t.append(wt)

                yst = ystp.tile([P, nsub, C], bf16, tag="yst")
                if level < 3:
                    continue
                for (woff, W) in _tiles(Se):
                    # layer 1: out [h, slots]
                    hs = []
                    for hk in range(HK):
                        wrow = w1p.tile([P, CK * P], bf16, tag="w1r")
                        nc.sync.dma_start(wrow[:], w1[ei, hk])
                        ps = p1.tile([P, W], f32, space="PSUM", tag="ps1")
                        for k in range(CK):
                            nc.tensor.matmul(
                                ps[:], lhsT=wrow[:, k * P:(k + 1) * P],
                                rhs=xg[:, k, woff:woff + W],
                                start=(k == 0), stop=(k == CK - 1))
                        ht = hp.tile([P, W], bf16, tag=f"h{hk}")
                        nc.scalar.activation(ht[:], ps[:], AF.Relu,
                                             bias=b1e[:, hk:hk + 1])
                        hs.append(ht)
                    if level < 4:
                        continue
                    # layer 2: h stationary -> out token-major [slots, C]
                    for sub in range((W + 127) // 128):
                        lo = sub * 128
                        wsub = min(128, W - lo)
                        gsub = (woff + lo) // 128
                        psA = p2.tile([P, 512], f32, space="PSUM", tag="ps2a")
                        psB = p2.tile([P, 512], f32, space="PSUM", tag="ps2b")
                        for hk in range(HK):
                            nc.tensor.matmul(
                                psA[0:wsub, :],
                                lhsT=hs[hk][:, lo:lo + wsub],
                                rhs=w2t[hk][:, 0:512],
                                start=(hk == 0), stop=(hk == HK - 1))
                            nc.tensor.matmul(
                                psB[0:wsub, :],
                                lhsT=hs[hk][:, lo:lo + wsub],
                                rhs=w2t[hk][:, 512:1024],
                                start=(hk == 0), stop=(hk == HK - 1))
                        nc.vector.tensor_tensor(
                            out=yst[0:wsub, gsub, 0:512], in0=psA[0:wsub, :],
                            in1=ggt[0:wsub, gsub, 0:1]
                            .to_broadcast([wsub, 512]), op=ALU.mult)
                        nc.vector.tensor_tensor(
                            out=yst[0:wsub, gsub, 512:1024],
                            in0=psB[0:wsub, :],
                            in1=ggt[0:wsub, gsub, 0:1]
                            .to_broadcast([wsub, 512]), op=ALU.mult)
                if level < 5:
                    continue
                # ---- combine: scatter-add token rows into yT ----
                nc.gpsimd.dma_scatter_add(yT, yst[:], t16T, Se, Se, C,
                                          queue_num=1)

        # software pipeline: stage A of rep+1 is emitted before stage B of
        # rep so the scheduler overlaps the routing chain with PE work
        actx = stage_a(0)
        for rep in range(repeat):
            nxt = stage_a(rep + 1) if rep + 1 < repeat else None
            if level >= 2 and actx is not None:
                stage_b(rep, actx)
            actx = nxt

    return nc


# ---------------- host side ----------------

def _host_caps(xf, gate_w, gate_b, ntok=NTOK, margin=16):
    """Slot capacities per expert from a host replica of the routing."""
    logits = xf.astype(np.float32) @ gate_w.astype(np.float32) + gate_b
    order = np.argpartition(-logits, TOPK - 1, axis=1)[:, :TOPK]
    ncore = xf.shape[0] // ntok
    counts = np.zeros((ncore, E), np.int64)
    for cc in range(ncore):
        sl = order[cc * ntok:(cc + 1) * ntok]
        counts[cc] = np.bincount(sl.ravel(), minlength=E)
    maxc = counts.max(axis=0)
    S = ((maxc + margin + 15) // 16) * 16
    assert S.max() <= 512, f"capacity overflow: {S}"
    return S.astype(np.int64)


def kernel(x, gate_w, gate_b, w1, b1, w2, b2):
    from concourse.bass_utils import run_bass_kernel_spmd
    import ml_dtypes

    x = np.asarray(x, np.float32)
    gate_w = np.asarray(gate_w, np.float32)
    gate_b = np.asarray(gate_b, np.float32)
    w1 = np.asarray(w1, np.float32)
    b1 = np.asarray(b1, np.float32)
    w2 = np.asarray(w2, np.float32)
    b2 = np.asarray(b2, np.float32)

    # w1 in lhsT-chunk layout: [E, HK, P(c in chunk), CK*P(h)]
    w1r = np.ascontiguousarray(
        (w1.reshape(E, CK, P, HK, P).transpose(0, 3, 2, 1, 4)
         .reshape(E, HK, P, C)).astype(ml_dtypes.bfloat16))
    w2b = np.ascontiguousarray(w2.astype(ml_dtypes.bfloat16))

    b, t, c = x.shape
    xf = x.reshape(b * t, c)
    S = _host_caps(xf, gate_w, gate_b)
    nc = build_program(S)

    shared = {
        "gw": gate_w,
        "gb": gate_b.reshape(E, 1).copy(),
        "w1": w1r,
        "b1": b1.reshape(E, H, 1).copy(),
        "w2": w2b,
        "b2e": b2,
        "id8": np.eye(E, dtype=np.float32),
        "id128": np.eye(P, dtype=np.float32),
    }
    in_maps = []
    for cc in range(NCORE):
        sl = xf[cc * NTOK:(cc + 1) * NTOK]
        m = dict(shared)
        m["xT"] = np.ascontiguousarray(sl.T)
        m["xtm"] = np.ascontiguousarray(sl.astype(ml_dtypes.bfloat16))
        in_maps.append(m)

    global LAST_BUILD, LAST_S
    LAST_BUILD = (nc, in_maps)
    LAST_S = S
    res = run_bass_kernel_spmd(nc, in_maps, core_ids=list(range(NCORE)))
    outs = [np.asarray(r["yT"][:NTOK]).astype(np.float32)
            for r in res.results]
    y = np.concatenate(outs, axis=0).reshape(b, t, c)
    return y



# revision 24
# speedup vs baseline: 1.2650x; 1.2650x over previous
"""Trainium2 Bass kernel for top-2 MoE (nn_ExpertMemory) — expert parallel.

Model (reference semantics):
    logits = x @ gate_w + gate_b          # (N, E)
    probs  = softmax(logits)
    gates, idx = top_k(probs, 2)
    out[n] = sum_k gates[n,k] * (relu(x[n] @ w1[e] + b1[e]) @ w2[e] + b2[e]),
             e = idx[n,k]

Sharding: expert parallelism. Core c owns expert c; its w1/w2 (8 MB bf16)
stay RESIDENT in SBUF across iterations, eliminating the 64 MB/core
weight streaming of the data-parallel layout. Tokens are data-parallel
for routing only: core j ("home" of tokens [1024j, 1024j+1024)) computes
the fp32 gate + top-2 for its tokens and AllGathers a compact payload:
2048 gate rows (row 2t+r, 64-wide) + 128 candidate rows encoding
candU = t + 1 + 16384*r for each (expert, token) pick (else -1).
Each expert core then:
  1. reads all 8 homes' candidate arrays (strided [16,512] views), masks
     to its own expert via a one-hot input (emask), rebases to global
     token ids, and runs ONE sparse_gather over the [16, 4096] concat ->
     tight-packed global slot list; home runs stay contiguous and in
     home-local order because sparse_gather traverses f-major (f*16+p)
  2. dma_gather(transpose) of token rows from the REPLICATED full x
     (xtm[8192, C] bf16 on every core; dispatch reads are local) and of
     gate rows from the AllGather output (row 2*t_g + r + 128*home)
  3. 2-layer MLP from SBUF-resident weights, in two slot halves; gate
     applied on the PSUM->SBUF copy
  4. dma_scatter_add of rows into a zeroed AllToAll input at row
     home*CAP + pos-within-home-run (pads get index -1: trailing
     negatives are skipped by the scatter)
  5. AllToAll returns each home its tokens' expert rows; the home
     scatter-adds region e rows into yT (pre-initialized with the
     gate-weighted b2 correction) using its own per-expert token lists,
     whose order matches the expert core's runs by construction.
All staging is double-buffered by repeat parity; stage A of rep i+1 and
its AllGather are emitted before stage E of rep i so routing and
collectives overlap the expert MLP.
"""

import numpy as np
from contextlib import ExitStack

import concourse.bass as bass
import concourse.tile as tile
import concourse.mybir as mybir
from concourse import bacc

dt = mybir.dt
AF = mybir.ActivationFunctionType
ALU = mybir.AluOpType
AX = mybir.AxisListType

P = 128

# problem constants
B, T, C, E, H, TOPK = 4, 2048, 1024, 8, 2048, 2
NCORE = 8
NTOK = B * T // NCORE   # tokens per home core (1024)
NTOT = B * T            # all tokens (8192)
TCH = NTOK // P         # token chunks per home (8)
CK = C // P             # C chunks (8)
HK = H // P             # H chunks (16)
GW = 64                 # AllGather row width (64 f32 = 256 B)
NH = NTOK // 512        # 512-wide token halves for the gate matmul
RBIT = 16384.0          # rank-bit offset in candU encoding
AGG = 2 * NTOK          # gate rows in AG payload (2048)
AGC = E * NTOK // GW    # candidate rows (128)
AGR = AGG + AGC         # AG payload rows per rank (2176)
OFF_U = AGG * GW        # f32 offset of the candidate region


def _tiles(s):
    out = []
    off = 0
    rem = s
    while rem > 512:
        out.append((off, 512))
        off += 512
        rem -= 512
    if rem:
        out.append((off, rem))
    return out


def build_program(S, ntok=NTOK, level=9, repeat=1, debug=False):
    nc = _build(S, ntok=ntok, level=level, repeat=repeat, debug=debug)
    nc.compile()
    return nc


def _build(S, ntok=NTOK, level=9, repeat=1, debug=False):
    """S: (SLOT, CAP): SLOT = global per-expert slot capacity (multiple
    of 128, SLOT/16 <= 512); CAP = per-(expert, home) A2A region rows
    (multiple of 128)."""
    SLOT, CAP = int(S[0]), int(S[1])
    assert SLOT % 128 == 0 and SLOT // 16 <= 512
    assert CAP % 128 == 0
    S16 = SLOT // 16
    CAP16 = CAP // 16
    # slot chunks of <= 512 (transpose dma_gather num_idxs HW cap), each a
    # multiple of 128
    HALVES = _tiles(SLOT)

    nc = bacc.Bacc("TRN2", target_bir_lowering=False, debug=False,
                   num_swdge_queues=2)

    f32, bf16 = dt.float32, dt.bfloat16
    xT = nc.dram_tensor("xT", [C, ntok], f32, kind="ExternalInput").ap()
    xtm = nc.dram_tensor("xtm", [NTOT, C], bf16, kind="ExternalInput").ap()
    gw = nc.dram_tensor("gw", [C, E], f32, kind="ExternalInput").ap()
    gb = nc.dram_tensor("gb", [E, 1], f32, kind="ExternalInput").ap()
    w1 = nc.dram_tensor("w1", [HK, P, CK * P], bf16,
                        kind="ExternalInput").ap()
    b1 = nc.dram_tensor("b1", [H, 1], f32, kind="ExternalInput").ap()
    w2 = nc.dram_tensor("w2", [HK, P, C], bf16, kind="ExternalInput").ap()
    b2e = nc.dram_tensor("b2e", [E, C], f32, kind="ExternalInput").ap()
    emask = nc.dram_tensor("emask", [16, E], f32, kind="ExternalInput").ap()
    id8 = nc.dram_tensor("id8", [E, E], f32, kind="ExternalInput").ap()
    id128 = nc.dram_tensor("id128", [P, P], f32, kind="ExternalInput").ap()
    yT = nc.dram_tensor("yT", [ntok + 16, C], bf16, kind="ExternalOutput").ap()

    # staging, double-buffered by repeat parity
    agin = nc.dram_tensor("agin", [2, AGR, GW], f32).ap()
    agout = nc.dram_tensor("agout", [2, NCORE, AGR, GW], f32).ap()
    a2in = nc.dram_tensor("a2in", [2, NCORE * CAP + 16, C], bf16).ap()
    a2out = nc.dram_tensor("a2out", [2, NCORE, CAP, C], bf16).ap()
    ycsta = nc.dram_tensor("ycsta", [2, ntok, C], bf16).ap()
    if debug:
        S16d = int(S[0]) // 16
        dbgA = nc.dram_tensor("dbgA", [4, 16, S16d], f32,
                              kind="ExternalOutput").ap()
        dbgC = nc.dram_tensor("dbgC", [3, 16, NCORE], f32,
                              kind="ExternalOutput").ap()
        dbgH = nc.dram_tensor("dbgH", [NCORE, 16, int(S[1]) // 16], f32,
                              kind="ExternalOutput").ap()
    rg = [list(range(NCORE))]

    with tile.TileContext(nc) as tc, ExitStack() as ctx:
        cpool = ctx.enter_context(tc.tile_pool(name="const", bufs=1))
        gpool = ctx.enter_context(tc.tile_pool(name="gk", bufs=2))
        sa = ctx.enter_context(tc.tile_pool(name="sa", bufs=2))
        sa1 = ctx.enter_context(tc.tile_pool(name="sa1", bufs=2))
        xtp = ctx.enter_context(tc.tile_pool(name="xt", bufs=2))
        mb = ctx.enter_context(tc.tile_pool(name="mb", bufs=1))
        xgp = ctx.enter_context(tc.tile_pool(name="xgp", bufs=1))
        ggp = ctx.enter_context(tc.tile_pool(name="ggp", bufs=1))
        hp = ctx.enter_context(tc.tile_pool(name="hp", bufs=1))
        ystp = ctx.enter_context(tc.tile_pool(name="ystp", bufs=1))
        ycp = ctx.enter_context(tc.tile_pool(name="ycp", bufs=2))
        cbp = ctx.enter_context(tc.tile_pool(name="cbp", bufs=2))
        pgp = ctx.enter_context(tc.tile_pool(name="pgp", bufs=1,
                                             space="PSUM"))
        pms = ctx.enter_context(tc.tile_pool(name="pms", bufs=1,
                                             space="PSUM"))
        p1 = ctx.enter_context(tc.tile_pool(name="p1", bufs=2, space="PSUM"))
        p2 = ctx.enter_context(tc.tile_pool(name="p2", bufs=2, space="PSUM"))

        # ---- constants (loaded once; weights resident) ----
        gwsb = cpool.tile([P, CK * E], f32)
        nc.sync.dma_start(gwsb[:].rearrange("p (k e) -> p k e", e=E),
                          gw.rearrange("(k p) e -> p k e", p=P))
        id8sb = cpool.tile([E, E], f32)
        nc.sync.dma_start(id8sb[:], id8)
        id128sb = cpool.tile([P, P], f32)
        nc.sync.dma_start(id128sb[:], id128)
        gbsb = cpool.tile([E, 1], f32)
        nc.sync.dma_start(gbsb[:], gb)
        b2sb = cpool.tile([E, C], f32)
        nc.sync.dma_start(b2sb[:], b2e)
        emsb = cpool.tile([16, E], f32)
        nc.sync.dma_start(emsb[:], emask)
        emb = cpool.tile([16, E, GW], f32)
        nc.vector.tensor_scalar_add(emb[:],
                                    emsb[:].to_broadcast([16, E, GW]), 0.0)
        b1sb = cpool.tile([P, HK], f32)
        nc.scalar.dma_start(b1sb[:].rearrange("p (k o) -> p k o", o=1),
                            b1.rearrange("(k p) one -> p k one", p=P))
        w1t = []
        for hk in range(HK):
            wt = cpool.tile([P, CK * P], bf16, tag=f"w1_{hk}")
            nc.sync.dma_start(wt[:], w1[hk])
            w1t.append(wt)
        w2t = []
        for hk in range(HK):
            wt = cpool.tile([P, C], bf16, tag=f"w2_{hk}")
            nc.sync.dma_start(wt[:], w2[hk])
            w2t.append(wt)

        iotaE_i = cpool.tile([P, TCH * E], dt.int32)
        nc.gpsimd.iota(iotaE_i[:], pattern=[[0, TCH], [1, E]], base=0,
                       channel_multiplier=0)
        iotaE = cpool.tile([P, TCH * E], f32)
        nc.vector.tensor_copy(iotaE[:], iotaE_i[:])
        toks_i = cpool.tile([P, TCH], dt.int32)
        nc.gpsimd.iota(toks_i[:], pattern=[[P, TCH]], base=0,
                       channel_multiplier=1)
        toksf = cpool.tile([P, TCH], f32)
        nc.vector.tensor_copy(toksf[:], toks_i[:])
        NIO = max(S16, CAP16)
        slotio_i = cpool.tile([16, NIO], dt.int32)
        nc.gpsimd.iota(slotio_i[:], pattern=[[16, NIO]], base=0,
                       channel_multiplier=1)
        slotio = cpool.tile([16, NIO], f32)
        nc.vector.tensor_copy(slotio[:], slotio_i[:])
        io8_i = cpool.tile([16, E], dt.int32)
        nc.gpsimd.iota(io8_i[:], pattern=[[1, E]], base=0,
                       channel_multiplier=0)
        io8 = cpool.tile([16, E], f32)
        nc.vector.tensor_copy(io8[:], io8_i[:])
        ones16 = cpool.tile([P, 16], f32)
        nc.vector.memset(ones16[:], 1.0)
        o16s = cpool.tile([16, 16], f32)
        nc.vector.memset(o16s[:], 1.0)
        zt = cpool.tile([P, 1024], bf16)
        nc.vector.memset(zt[:], 0.0)

        def stage_a(rep):
            """Home-side: gate + top-2 routing on own 1024 tokens; write
            the AllGather payload + b2 correction + per-expert counts."""
            par = rep % 2
            flatA = agin[par].rearrange("r w -> (r w)")

            # gate logits, expert-major (true fp32: fp32r flips top-2 picks)
            lgT = sa1.tile([E, ntok], f32, tag="big8")
            for nh in range(NH):
                lgps = pgp.tile([E, 512], f32, space="PSUM", tag="lgp")
                for k in range(CK):
                    xt = xtp.tile([P, 512], f32, tag="xt")
                    nc.sync.dma_start(
                        xt[:], xT[k * P:(k + 1) * P,
                                  nh * 512:(nh + 1) * 512])
                    nc.tensor.matmul(lgps[:],
                                     lhsT=gwsb[:, k * E:(k + 1) * E],
                                     rhs=xt[:],
                                     start=(k == 0), stop=(k == CK - 1))
                nc.vector.tensor_scalar_add(lgT[:, nh * 512:(nh + 1) * 512],
                                            lgps[:], gbsb[:, :1])
            lg = sa1.tile([P, TCH, E], f32, tag="lg")
            for t in range(TCH):
                ps = pms.tile([P, E], f32, space="PSUM", tag="misc")
                nc.tensor.transpose(ps[:], lgT[:, t * P:(t + 1) * P],
                                    id8sb[:])
                nc.vector.tensor_copy(lg[:, t, :], ps[:])
            # softmax over experts
            mx = sa.tile([P, TCH], f32, tag="mx")
            nc.vector.tensor_reduce(mx[:], lg[:], axis=AX.X, op=ALU.max)
            xm = sa.tile([P, TCH, E], f32, tag="xm")
            nc.vector.tensor_tensor(out=xm[:], in0=lg[:],
                                    in1=mx[:].to_broadcast([P, TCH, E]),
                                    op=ALU.subtract)
            ex = sa.tile([P, TCH, E], f32, tag="ex")
            nc.scalar.activation(ex[:], xm[:], AF.Exp)
            sm = sa.tile([P, TCH], f32, tag="sm")
            nc.vector.tensor_reduce(sm[:], ex[:], axis=AX.X, op=ALU.add)
            rs = sa.tile([P, TCH], f32, tag="rs")
            nc.vector.reciprocal(rs[:], sm[:])
            probs = sa.tile([P, TCH, E], f32, tag="probs")
            nc.vector.tensor_tensor(out=probs[:], in0=ex[:],
                                    in1=rs[:].to_broadcast([P, TCH, E]),
                                    op=ALU.mult)
            # top-2 by logits
            mig = sa.tile([P, TCH, 8], dt.uint32, tag="mig")
            for t in range(TCH):
                mv = sa.tile([P, 8], f32, tag="mv")
                nc.vector.max(mv[:], lg[:, t, :])
                nc.vector.max_index(mig[:, t, :], mv[:], lg[:, t, :])
            migf = sa.tile([P, TCH, 8], f32, tag="migf")
            nc.vector.tensor_copy(migf[:], mig[:])

            A = []
            g = []
            for r in range(2):
                Ar = sa1.tile([P, TCH, E], f32, tag=f"A{r}")
                nc.vector.tensor_tensor(
                    out=Ar[:],
                    in0=migf[:, :, r:r + 1].to_broadcast([P, TCH, E]),
                    in1=iotaE[:].rearrange("p (t e) -> p t e", e=E),
                    op=ALU.is_equal)
                gr = gpool.tile([P, TCH], f32, tag=f"g{r}")
                tmp = sa.tile([P, TCH, E], f32, tag="gt")
                nc.vector.tensor_tensor(out=tmp[:], in0=probs[:], in1=Ar[:],
                                        op=ALU.mult)
                nc.vector.tensor_reduce(gr[:], tmp[:], axis=AX.X, op=ALU.add)
                A.append(Ar)
                g.append(gr)
            M = sa1.tile([P, TCH, E], f32, tag="M")
            nc.vector.tensor_tensor(out=M[:], in0=A[0][:], in1=A[1][:],
                                    op=ALU.add)

            # per-expert token counts on 16 partitions (junk-tail masking)
            Mre = sa.tile([P, E, TCH], f32, tag="Mre")
            nc.vector.tensor_copy(Mre[:], M[:].rearrange("p t e -> p e t"))
            cntp = pms.tile([16, E * TCH], f32, space="PSUM", tag="misc")
            nc.tensor.matmul(cntp[:], lhsT=ones16[:],
                             rhs=Mre[:].rearrange("p e t -> p (e t)"),
                             start=True, stop=True)
            cntet = sa.tile([16, E, TCH], f32, tag="cntet")
            nc.vector.tensor_copy(cntet[:],
                                  cntp[:].rearrange("p (e t) -> p e t", e=E))
            cnt16 = gpool.tile([16, E], f32, tag="cnt16")
            nc.vector.tensor_reduce(cnt16[:], cntet[:], axis=AX.X, op=ALU.add)

            # candidate encoding: candU = t + 1 + RBIT*r (else -1)
            tokp1 = sa.tile([P, TCH], f32, tag="tokp1")
            nc.vector.tensor_scalar_add(tokp1[:], toksf[:], 2.0)
            base = sa.tile([P, TCH, E], f32, tag="base")
            nc.vector.scalar_tensor_tensor(
                out=base[:], in0=A[1][:], scalar=RBIT,
                in1=tokp1[:].to_broadcast([P, TCH, E]),
                op0=ALU.mult, op1=ALU.add)
            candU = sa1.tile([P, TCH, E], f32, tag="candU")
            nc.vector.tensor_tensor(out=candU[:], in0=base[:], in1=M[:],
                                    op=ALU.mult)
            nc.vector.tensor_scalar_add(candU[:], candU[:], -1.0)
            for ei in range(E):
                nc.scalar.dma_start(
                    flatA[OFF_U + ei * ntok:OFF_U + (ei + 1) * ntok]
                    .rearrange("(t p) -> p t", p=P),
                    candU[:, :, ei])

            # gate rows 2t+r of the payload
            for r in range(2):
                g64 = sa.tile([P, TCH, GW], f32, tag="g64")
                nc.vector.tensor_scalar_add(
                    g64[:], g[r][:].to_broadcast([P, TCH, GW]), 0.0)
                nc.scalar.dma_start(
                    agin[par, 0:AGG, :].rearrange(
                        "(tch p two) f -> p tch two f",
                        p=P, two=2)[:, :, r, :],
                    g64[:])

            # b2 correction sum_r g_r * b2[e_r] -> ycsta (DRAM staging)
            wtok = sa1.tile([P, TCH, E], f32, tag="wtok")
            nc.vector.tensor_tensor(
                out=wtok[:], in0=A[0][:],
                in1=g[0][:].to_broadcast([P, TCH, E]), op=ALU.mult)
            wtk1 = sa.tile([P, TCH, E], f32, tag="wtk1")
            nc.vector.tensor_tensor(
                out=wtk1[:], in0=A[1][:],
                in1=g[1][:].to_broadcast([P, TCH, E]), op=ALU.mult)
            nc.vector.tensor_tensor(out=wtok[:], in0=wtok[:], in1=wtk1[:],
                                    op=ALU.add)
            wTe = sa1.tile([E, TCH * P], f32, tag="big8")
            for t in range(TCH):
                pw = pms.tile([E, P], f32, space="PSUM", tag="misc")
                nc.tensor.transpose(pw[:], wtok[:, t, :], id128sb[:])
                nc.vector.tensor_copy(wTe[:, t * P:(t + 1) * P], pw[:])
            for t in range(TCH):
                yc = ycp.tile([P, C], bf16, tag="yc")
                for hh in range(2):
                    pc = pms.tile([P, 512], f32, space="PSUM", tag="misc")
                    nc.tensor.matmul(pc[:], lhsT=wTe[:, t * P:(t + 1) * P],
                                     rhs=b2sb[:, hh * 512:(hh + 1) * 512],
                                     start=True, stop=True)
                    nc.vector.tensor_copy(yc[:, hh * 512:(hh + 1) * 512],
                                          pc[:])
                nc.scalar.dma_start(
                    ycsta[par, t * P:(t + 1) * P, :], yc[:])
            return dict(cnt16=cnt16)

        def do_ag(rep):
            par = rep % 2
            nc.gpsimd.collective_compute(
                "AllGather", ALU.bypass, replica_groups=rg,
                ins=[agin[par]],
                outs=[agout[par].rearrange("n r w -> (n r) w")])

        def do_a2a(rep):
            par = rep % 2
            nc.gpsimd.collective_compute(
                "AllToAll", ALU.bypass, replica_groups=rg,
                ins=[a2in[par, 0:NCORE * CAP]
                     .rearrange("(n s) c -> n s c", n=NCORE)],
                outs=[a2out[par]])

        def stage_e(rep):
            """Expert-side: build own-expert global slot list, gather,
            MLP from resident weights, scatter into the A2A input."""
            par = rep % 2
            # zero the A2A input (scatter_add needs a clean base)
            a2flat = a2in[par].rearrange("s c -> (s c)")
            zchunk = P * 1024
            nz = NCORE * CAP * C // zchunk
            for z in range(nz):
                nc.sync.dma_start(
                    a2flat[z * zchunk:(z + 1) * zchunk]
                    .rearrange("(p f) -> p f", p=P),
                    zt[:])

            flatO = agout[par].rearrange("n r w -> (n r w)")
            ctU = mb.tile([16, NCORE * 512], f32, tag="ctU")
            cpart = mb.tile([16, NCORE], f32, tag="cpart")
            for j in range(NCORE):
                rU = mb.tile([16, 512], f32, tag="rU")
                nc.scalar.dma_start(
                    rU[:], flatO[j * AGR * GW + OFF_U:
                                 j * AGR * GW + OFF_U + E * ntok]
                    .rearrange("(f p) -> p f", p=16))
                # mask to own expert (others/pads -> -1), then rebase the
                # valid entries only: + j*ntok - 1 (u encodes t_local+1)
                uj = ctU[:, j * 512:(j + 1) * 512]
                nc.vector.scalar_tensor_tensor(
                    out=uj, in0=rU[:], scalar=1.0,
                    in1=emb[:].rearrange("p e w -> p (e w)"),
                    op0=ALU.add, op1=ALU.mult)
                nc.vector.tensor_scalar_add(uj, uj, -1.0)
                vm = mb.tile([16, 512], f32, tag="vm")
                nc.vector.tensor_scalar(vm[:], uj, 0.0, 0.0,
                                        op0=ALU.is_ge, op1=ALU.add)
                nc.vector.scalar_tensor_tensor(
                    out=uj, in0=vm[:], scalar=float(j * ntok - 1),
                    in1=uj, op0=ALU.mult, op1=ALU.add)
                nc.vector.tensor_reduce(cpart[:, j:j + 1], vm[:],
                                        axis=AX.X, op=ALU.add)

            # per-home counts -> A2A row offsets off[j] = j*CAP - start_j
            chp = pms.tile([16, NCORE], f32, space="PSUM", tag="misc")
            nc.tensor.matmul(chp[:], lhsT=o16s[:], rhs=cpart[:],
                             start=True, stop=True)
            ch = mb.tile([16, NCORE], f32, tag="ch")
            nc.vector.tensor_copy(ch[:], chp[:])

            # two-level compaction (sparse_gather input free dim is capped
            # at 512 on HW): per-home 512 -> CAP16 (junk tail -> -1 via the
            # home count), then one gather over the padded concat.
            r2in = mb.tile([16, NCORE * CAP16], f32, tag="r2in")
            for j in range(NCORE):
                r1 = mb.tile([16, CAP16], f32, tag="r1")
                nc.vector.memset(r1[:], -1.0)
                nfd1 = mb.tile([1, 1], dt.uint32, tag="nfd1")
                nc.gpsimd.sparse_gather(
                    r1[:], ctU[:, j * 512:(j + 1) * 512],
                    num_found=nfd1[:])
                mj = mb.tile([16, CAP16], f32, tag="mj")
                nc.vector.tensor_tensor(
                    out=mj[:], in0=slotio[:, :CAP16],
                    in1=ch[:, j:j + 1].to_broadcast([16, CAP16]),
                    op=ALU.is_lt)
                nc.vector.tensor_scalar_add(r1[:], r1[:], 1.0)
                nc.vector.tensor_tensor(out=r1[:], in0=r1[:], in1=mj[:],
                                        op=ALU.mult)
                nc.vector.tensor_scalar_add(
                    r2in[:, j * CAP16:(j + 1) * CAP16], r1[:], -1.0)
            tkU = mb.tile([16, S16], f32, tag="tkU")
            nc.vector.memset(tkU[:], -1.0)
            nfd = mb.tile([1, 1], dt.uint32, tag="nfd")
            nc.gpsimd.sparse_gather(tkU[:], r2in[:], num_found=nfd[:])
            pf = ch
            for sh in (1, 2, 4):
                nxtp = mb.tile([16, NCORE], f32, tag=f"pf{sh}")
                nc.vector.tensor_copy(nxtp[:, 0:sh], pf[:, 0:sh])
                nc.vector.tensor_tensor(out=nxtp[:, sh:NCORE],
                                        in0=pf[:, sh:NCORE],
                                        in1=pf[:, 0:NCORE - sh], op=ALU.add)
                pf = nxtp
            excl = mb.tile([16, NCORE], f32, tag="excl")
            nc.vector.tensor_tensor(out=excl[:], in0=pf[:], in1=ch[:],
                                    op=ALU.subtract)
            off = mb.tile([16, NCORE], f32, tag="off")
            nc.vector.scalar_tensor_tensor(
                out=off[:], in0=io8[:], scalar=float(CAP), in1=excl[:],
                op0=ALU.mult, op1=ALU.subtract)
            total = pf[:, NCORE - 1:NCORE]

            # decode: rank bit, global token id (clamped), home, gate row
            rb = mb.tile([16, S16], f32, tag="rb")
            nc.vector.tensor_scalar(rb[:], tkU[:], RBIT, 0.0,
                                    op0=ALU.is_ge, op1=ALU.add)
            xf = mb.tile([16, S16], f32, tag="xf")
            nc.vector.scalar_tensor_tensor(
                out=xf[:], in0=rb[:], scalar=-RBIT, in1=tkU[:],
                op0=ALU.mult, op1=ALU.add)
            nc.vector.tensor_scalar(xf[:], xf[:], 0.0, float(NTOT - 1),
                                    op0=ALU.max, op1=ALU.min)
            js = mb.tile([16, S16], f32, tag="js")
            nc.vector.tensor_scalar(js[:], xf[:], float(ntok), 0.0,
                                    op0=ALU.is_ge, op1=ALU.add)
            for k in range(2, NCORE):
                jtmp = mb.tile([16, S16], f32, tag="jtmp")
                nc.vector.tensor_scalar(jtmp[:], xf[:], float(k * ntok),
                                        0.0, op0=ALU.is_ge, op1=ALU.add)
                nc.vector.tensor_tensor(out=js[:], in0=js[:], in1=jtmp[:],
                                        op=ALU.add)
            # gate row = 2*t_g + r + (AGR-2048)*home
            grow = mb.tile([16, S16], f32, tag="grow")
            nc.vector.scalar_tensor_tensor(
                out=grow[:], in0=xf[:], scalar=2.0, in1=rb[:],
                op0=ALU.mult, op1=ALU.add)
            gtmp = mb.tile([16, S16], f32, tag="gtmp")
            nc.vector.scalar_tensor_tensor(
                out=gtmp[:], in0=js[:], scalar=float(AGR - AGG), in1=grow[:],
                op0=ALU.mult, op1=ALU.add)
            # scatter row = slot + off[home]; pads (slot >= total) -> -1
            osel = mb.tile([16, S16], f32, tag="osel")
            nc.vector.memset(osel[:], 0.0)
            for k in range(NCORE):
                ieq = mb.tile([16, S16], f32, tag="ieq")
                nc.vector.tensor_scalar(ieq[:], js[:], float(k), 0.0,
                                        op0=ALU.is_equal, op1=ALU.add)
                nc.vector.tensor_tensor(
                    out=ieq[:], in0=ieq[:],
                    in1=off[:, k:k + 1].to_broadcast([16, S16]),
                    op=ALU.mult)
                nc.vector.tensor_tensor(out=osel[:], in0=osel[:],
                                        in1=ieq[:], op=ALU.add)
            scat = mb.tile([16, S16], f32, tag="scat")
            nc.vector.tensor_tensor(out=scat[:], in0=slotio[:, :S16],
                                    in1=osel[:], op=ALU.add)
            # junk tail -> trash row NCORE*CAP (payload junk collides only
            # in trash; skipped/negative indices hang the SWDGE scatter)
            msk = mb.tile([16, S16], f32, tag="msk")
            nc.vector.tensor_tensor(out=msk[:], in0=slotio[:, :S16],
                                    in1=total.to_broadcast([16, S16]),
                                    op=ALU.is_lt)
            nc.vector.tensor_scalar_add(scat[:], scat[:],
                                        float(-NCORE * CAP))
            nc.vector.tensor_tensor(out=scat[:], in0=scat[:], in1=msk[:],
                                    op=ALU.mult)
            nc.vector.tensor_scalar_add(scat[:], scat[:],
                                        float(NCORE * CAP))

            if debug and rep == 0:
                nc.scalar.dma_start(dbgA[0], tkU[:])
                nc.scalar.dma_start(dbgA[1], xf[:])
                nc.scalar.dma_start(dbgA[2], scat[:])
                nc.scalar.dma_start(dbgA[3], gtmp[:])
                nc.scalar.dma_start(dbgC[0], ch[:])
                nc.scalar.dma_start(dbgC[1], off[:])
                nc.scalar.dma_start(dbgC[2], cpart[:])
            trip = mb.tile([P, 3 * S16], dt.int16, tag="trip")
            nc.vector.tensor_copy(trip[0:16, 0:S16], xf[:])
            nc.vector.tensor_copy(trip[0:16, S16:2 * S16], scat[:])
            nc.vector.tensor_copy(trip[0:16, 2 * S16:3 * S16], gtmp[:])
            for sz in (16, 32, 64):
                nc.scalar.dma_start(trip[sz:2 * sz, :], trip[0:sz, :])
            t16X = trip[:, 0:S16]
            t16S = trip[:, S16:2 * S16]
            t16G = trip[:, 2 * S16:3 * S16]

            if level < 3:
                return
            for (hoff, W) in HALVES:
                h16 = hoff // 16
                ggt = ggp.tile([P, W // 128, GW], f32, tag="gg")
                nc.gpsimd.dma_gather(
                    ggt[:], agout[par].rearrange("n r w -> (n r) w"),
                    t16G[:, h16:h16 + W // 16], W, W, GW)
                if level < 4:
                    continue
                xg = xgp.tile([P, CK, W], bf16, tag="xg")
                nc.gpsimd.dma_gather(xg[:], xtm,
                                     t16X[:, h16:h16 + W // 16],
                                     W, W, C, transpose=True)
                if level < 5:
                    continue
                hs = []
                for hk in range(HK):
                    ps = p1.tile([P, W], f32, space="PSUM", tag="ps1")
                    for k in range(CK):
                        nc.tensor.matmul(
                            ps[:], lhsT=w1t[hk][:, k * P:(k + 1) * P],
                            rhs=xg[:, k, :],
                            start=(k == 0), stop=(k == CK - 1))
                    ht = hp.tile([P, W], bf16, tag=f"h{hk}")
                    nc.scalar.activation(ht[:], ps[:], AF.Relu,
                                         bias=b1sb[:, hk:hk + 1])
                    hs.append(ht)
                if level < 6:
                    continue
                yst = ystp.tile([P, W // 128, C], bf16, tag="yst")
                for sub in range(W // 128):
                    lo = sub * 128
                    psA = p2.tile([P, 512], f32, space="PSUM",
                                  tag="ps2a")
                    psB = p2.tile([P, 512], f32, space="PSUM",
                                  tag="ps2b")
                    for hk in range(HK):
                        nc.tensor.matmul(
                            psA[:], lhsT=hs[hk][:, lo:lo + 128],
                            rhs=w2t[hk][:, 0:512],
                            start=(hk == 0), stop=(hk == HK - 1))
                        nc.tensor.matmul(
                            psB[:], lhsT=hs[hk][:, lo:lo + 128],
                            rhs=w2t[hk][:, 512:1024],
                            start=(hk == 0), stop=(hk == HK - 1))
                    nc.vector.tensor_tensor(
                        out=yst[:, sub, 0:512], in0=psA[:],
                        in1=ggt[:, sub, 0:1].to_broadcast([P, 512]),
                        op=ALU.mult)
                    nc.vector.tensor_tensor(
                        out=yst[:, sub, 512:1024], in0=psB[:],
                        in1=ggt[:, sub, 0:1].to_broadcast([P, 512]),
                        op=ALU.mult)
                if level < 7:
                    continue
                nc.gpsimd.dma_scatter_add(
                    a2in[par], yst[:], t16S[:, h16:h16 + W // 16],
                    W, W, C, queue_num=1)

        def stage_c(rep, actx):
            """Home-side combine: yT = b2 correction + scatter-add of the
            returned expert rows (region e, home-local order)."""
            par = rep % 2
            cnt16 = actx["cnt16"]
            nc.sync.dma_start(yT[0:ntok, :], ycsta[par])
            flatA = agin[par].rearrange("r w -> (r w)")
            for e2 in range(NCORE):
                cw = cbp.tile([16, GW], f32, tag="cw")
                nc.scalar.dma_start(
                    cw[:], flatA[OFF_U + e2 * ntok:OFF_U + (e2 + 1) * ntok]
                    .rearrange("(f p) -> p f", p=16))
                msk = cbp.tile([16, CAP16], f32, tag="cmsk")
                nc.vector.tensor_tensor(
                    out=msk[:], in0=slotio[:, :CAP16],
                    in1=cnt16[:, e2:e2 + 1].to_broadcast([16, CAP16]),
                    op=ALU.is_lt)
                tk = cbp.tile([16, CAP16], f32, tag="ctk")
                nc.vector.memset(tk[:], -1.0)
                nfd = cbp.tile([1, 1], dt.uint32, tag="cnfd")
                nc.gpsimd.sparse_gather(tk[:, :CAP16], cw[:],
                                        num_found=nfd[:])
                # decode t+1 out of candU = t + 1 + RBIT*r
                rb = cbp.tile([16, CAP16], f32, tag="crb")
                nc.vector.tensor_scalar(rb[:], tk[:], RBIT, 0.0,
                                        op0=ALU.is_ge, op1=ALU.add)
                xff = cbp.tile([16, CAP16], f32, tag="cxff")
                nc.vector.scalar_tensor_tensor(
                    out=xff[:], in0=rb[:], scalar=-RBIT, in1=tk[:],
                    op0=ALU.mult, op1=ALU.add)
                nc.vector.tensor_scalar(xff[:], xff[:], 1.0, float(ntok),
                                        op0=ALU.max, op1=ALU.min)
                nc.vector.tensor_scalar_add(xff[:], xff[:], -1.0)
                stf = cbp.tile([16, CAP16], f32, tag="cstf")
                nc.vector.scalar_tensor_tensor(
                    out=stf[:], in0=xff[:], scalar=float(-ntok),
                    in1=msk[:], op0=ALU.add, op1=ALU.mult)
                nc.vector.tensor_scalar_add(stf[:], stf[:], float(ntok))
                if debug and rep == 0:
                    nc.scalar.dma_start(dbgH[e2], stf[:])
                t16 = cbp.tile([P, CAP16], dt.int16, tag="ct16")
                nc.vector.tensor_copy(t16[0:16, :], stf[:])
                for sz in (16, 32, 64):
                    nc.scalar.dma_start(t16[sz:2 * sz, :], t16[0:sz, :])
                bt = cbp.tile([P, CAP // 128, C], bf16, tag="bt")
                nc.sync.dma_start(
                    bt[:], a2out[par, e2].rearrange("(s p) c -> p s c",
                                                    p=P))
                nc.gpsimd.dma_scatter_add(yT, bt[:], t16[:], CAP, CAP, C,
                                          queue_num=1)

        # software pipeline: A(r+1) + AllGather before E(r)
        actxs = {0: stage_a(0)}
        if level >= 1:
            do_ag(0)
        for rep in range(repeat):
            if rep + 1 < repeat:
                actxs[rep + 1] = stage_a(rep + 1)
                if level >= 1:
                    do_ag(rep + 1)
            if level >= 2:
                stage_e(rep)
            if level >= 8:
                do_a2a(rep)
            if level >= 9:
                stage_c(rep, actxs.pop(rep))

    return nc


# ---------------- host side ----------------

def _host_route(xf, gate_w, gate_b):
    logits = xf.astype(np.float32) @ gate_w.astype(np.float32) + gate_b
    return np.argpartition(-logits, TOPK - 1, axis=1)[:, :TOPK]


def _host_caps(order, ntok=NTOK):
    cnt = np.bincount(order.ravel(), minlength=E)
    slot = int(np.ceil((cnt.max() + 16) / 128.0) * 128)
    ncore = order.shape[0] // ntok
    pair = np.zeros((E, ncore), np.int64)
    for j in range(ncore):
        sl = order[j * ntok:(j + 1) * ntok]
        pair[:, j] = np.bincount(sl.ravel(), minlength=E)
    cap = int(np.ceil((pair.max() + 16) / 128.0) * 128)
    assert slot // 16 <= 512, f"slot overflow: {slot}"
    return slot, cap


def kernel(x, gate_w, gate_b, w1, b1, w2, b2):
    from concourse.bass_utils import run_bass_kernel_spmd
    import ml_dtypes

    x = np.asarray(x, np.float32)
    gate_w = np.asarray(gate_w, np.float32)
    gate_b = np.asarray(gate_b, np.float32)
    w1 = np.asarray(w1, np.float32)
    b1 = np.asarray(b1, np.float32)
    w2 = np.asarray(w2, np.float32)
    b2 = np.asarray(b2, np.float32)

    # w1 in lhsT-chunk layout: [E, HK, P(c in chunk), CK*P(h)]
    w1r = np.ascontiguousarray(
        (w1.reshape(E, CK, P, HK, P).transpose(0, 3, 2, 1, 4)
         .reshape(E, HK, P, C)).astype(ml_dtypes.bfloat16))
    w2b = np.ascontiguousarray(
        w2.reshape(E, HK, P, C).astype(ml_dtypes.bfloat16))

    b, t, c = x.shape
    xf = x.reshape(b * t, c)
    order = _host_route(xf, gate_w, gate_b)
    S = _host_caps(order)
    nc = build_program(S)

    xtm_full = np.ascontiguousarray(xf.astype(ml_dtypes.bfloat16))
    shared = {
        "xtm": xtm_full,
        "gw": gate_w,
        "gb": gate_b.reshape(E, 1).copy(),
        "b2e": b2,
        "id8": np.eye(E, dtype=np.float32),
        "id128": np.eye(P, dtype=np.float32),
    }
    in_maps = []
    for cc in range(NCORE):
        sl = xf[cc * NTOK:(cc + 1) * NTOK]
        m = dict(shared)
        m["xT"] = np.ascontiguousarray(sl.T)
        m["w1"] = w1r[cc]
        m["b1"] = np.ascontiguousarray(b1[cc].reshape(H, 1))
        m["w2"] = w2b[cc]
        em = np.zeros((16, E), np.float32)
        em[:, cc] = 1.0
        m["emask"] = em
        in_maps.append(m)

    global LAST_BUILD, LAST_S
    LAST_BUILD = (nc, in_maps)
    LAST_S = S
    res = run_bass_kernel_spmd(nc, in_maps, core_ids=list(range(NCORE)))
    outs = [np.asarray(r["yT"][:NTOK]).astype(np.float32)
            for r in res.results]
    y = np.concatenate(outs, axis=0).reshape(b, t, c)
    return y


# revision 25
# speedup vs baseline: 2.0740x; 1.6396x over previous
"""Trainium2 Bass kernel for top-2 MoE (nn_ExpertMemory) — expert parallel.

Model (reference semantics):
    logits = x @ gate_w + gate_b          # (N, E)
    probs  = softmax(logits)
    gates, idx = top_k(probs, 2)
    out[n] = sum_k gates[n,k] * (relu(x[n] @ w1[e] + b1[e]) @ w2[e] + b2[e]),
             e = idx[n,k]

Sharding: expert parallelism. Core c owns expert c; its w1/w2 (8 MB bf16)
stay RESIDENT in SBUF across iterations, eliminating the 64 MB/core
weight streaming of the data-parallel layout. Tokens are data-parallel
for routing only: core j ("home" of tokens [1024j, 1024j+1024)) computes
the fp32 gate + top-2 for its tokens and AllGathers a compact payload:
2048 gate rows (row 2t+r, 64-wide) + 128 candidate rows encoding
candU = t + 1 + 16384*r for each (expert, token) pick (else -1).
Each expert core then:
  1. reads all 8 homes' candidate arrays (strided [16,512] views), masks
     to its own expert via a one-hot input (emask), rebases to global
     token ids, and runs ONE sparse_gather over the [16, 4096] concat ->
     tight-packed global slot list; home runs stay contiguous and in
     home-local order because sparse_gather traverses f-major (f*16+p)
  2. dma_gather(transpose) of token rows from the REPLICATED full x
     (xtm[8192, C] bf16 on every core; dispatch reads are local) and of
     gate rows from the AllGather output (row 2*t_g + r + 128*home)
  3. 2-layer MLP from SBUF-resident weights, in two slot halves; gate
     applied on the PSUM->SBUF copy
  4. dma_scatter_add of rows into a zeroed AllToAll input at row
     home*CAP + pos-within-home-run (pads get index -1: trailing
     negatives are skipped by the scatter)
  5. AllToAll returns each home its tokens' expert rows; the home
     scatter-adds region e rows into yT (pre-initialized with the
     gate-weighted b2 correction) using its own per-expert token lists,
     whose order matches the expert core's runs by construction.
All staging is double-buffered by repeat parity; stage A of rep i+1 and
its AllGather are emitted before stage E of rep i so routing and
collectives overlap the expert MLP.
"""

import numpy as np
from contextlib import ExitStack

import concourse.bass as bass
import concourse.tile as tile
import concourse.mybir as mybir
from concourse import bacc

dt = mybir.dt
AF = mybir.ActivationFunctionType
ALU = mybir.AluOpType
AX = mybir.AxisListType

P = 128

# problem constants
B, T, C, E, H, TOPK = 4, 2048, 1024, 8, 2048, 2
NCORE = 8
NTOK = B * T // NCORE   # tokens per home core (1024)
NTOT = B * T            # all tokens (8192)
TCH = NTOK // P         # token chunks per home (8)
CK = C // P             # C chunks (8)
HK = H // P             # H chunks (16)
GW = 64                 # AllGather row width (64 f32 = 256 B)
NH = NTOK // 512        # 512-wide token halves for the gate matmul
RBIT = 16384.0          # rank-bit offset in candU encoding
AGG = 2 * NTOK          # gate rows in AG payload (2048)
AGC = E * NTOK // GW    # candidate rows (128)
AGR = AGG + AGC         # AG payload rows per rank (2176)
OFF_U = AGG * GW        # f32 offset of the candidate region


def _tiles(s):
    out = []
    off = 0
    rem = s
    while rem > 512:
        out.append((off, 512))
        off += 512
        rem -= 512
    if rem:
        out.append((off, rem))
    return out


def build_program(S, ntok=NTOK, level=9, repeat=1, debug=False):
    nc = _build(S, ntok=ntok, level=level, repeat=repeat, debug=debug)
    nc.compile()
    return nc


def _build(S, ntok=NTOK, level=9, repeat=1, debug=False):
    """S: (SLOT, CAP): SLOT = global per-expert slot capacity (multiple
    of 128, SLOT/16 <= 512); CAP = per-(expert, home) A2A region rows
    (multiple of 128)."""
    SLOT, CAP = int(S[0]), int(S[1])
    assert SLOT % 128 == 0 and SLOT // 16 <= 512
    assert CAP % 128 == 0
    S16 = SLOT // 16
    CAP16 = CAP // 16
    # slot chunks of <= 512 (transpose dma_gather num_idxs HW cap), each a
    # multiple of 128
    HALVES = _tiles(SLOT)

    nc = bacc.Bacc("TRN2", target_bir_lowering=False, debug=False,
                   num_swdge_queues=2)

    f32, bf16 = dt.float32, dt.bfloat16
    xT = nc.dram_tensor("xT", [C, ntok], f32, kind="ExternalInput").ap()
    xtm = nc.dram_tensor("xtm", [NTOT, C], bf16, kind="ExternalInput").ap()
    gw = nc.dram_tensor("gw", [C, E], f32, kind="ExternalInput").ap()
    gb = nc.dram_tensor("gb", [E, 1], f32, kind="ExternalInput").ap()
    w1 = nc.dram_tensor("w1", [HK, P, CK * P], bf16,
                        kind="ExternalInput").ap()
    b1 = nc.dram_tensor("b1", [H, 1], f32, kind="ExternalInput").ap()
    w2 = nc.dram_tensor("w2", [HK, P, C], bf16, kind="ExternalInput").ap()
    b2e = nc.dram_tensor("b2e", [E, C], f32, kind="ExternalInput").ap()
    emask = nc.dram_tensor("emask", [16, E], f32, kind="ExternalInput").ap()
    id8 = nc.dram_tensor("id8", [E, E], f32, kind="ExternalInput").ap()
    id128 = nc.dram_tensor("id128", [P, P], f32, kind="ExternalInput").ap()
    yT = nc.dram_tensor("yT", [ntok + 16, C], bf16, kind="ExternalOutput").ap()

    # staging, double-buffered by repeat parity
    agin = nc.dram_tensor("agin", [2, AGR, GW], f32).ap()
    agout = nc.dram_tensor("agout", [2, NCORE, AGR, GW], f32).ap()
    a2in = nc.dram_tensor("a2in", [2, NCORE * CAP + 16, C], bf16).ap()
    a2out = nc.dram_tensor("a2out", [2, NCORE, CAP, C], bf16).ap()
    ycsta = nc.dram_tensor("ycsta", [2, ntok, C], bf16).ap()
    if debug:
        S16d = int(S[0]) // 16
        dbgA = nc.dram_tensor("dbgA", [4, 16, S16d], f32,
                              kind="ExternalOutput").ap()
        dbgC = nc.dram_tensor("dbgC", [3, 16, NCORE], f32,
                              kind="ExternalOutput").ap()
        dbgH = nc.dram_tensor("dbgH", [NCORE, 16, int(S[1]) // 16], f32,
                              kind="ExternalOutput").ap()
    rg = [list(range(NCORE))]

    with tile.TileContext(nc) as tc, ExitStack() as ctx:
        cpool = ctx.enter_context(tc.tile_pool(name="const", bufs=1))
        gpool = ctx.enter_context(tc.tile_pool(name="gk", bufs=2))
        sa = ctx.enter_context(tc.tile_pool(name="sa", bufs=2))
        sa1 = ctx.enter_context(tc.tile_pool(name="sa1", bufs=2))
        xtp = ctx.enter_context(tc.tile_pool(name="xt", bufs=2))
        mb = ctx.enter_context(tc.tile_pool(name="mb", bufs=1))
        xgp = ctx.enter_context(tc.tile_pool(name="xgp", bufs=1))
        ggp = ctx.enter_context(tc.tile_pool(name="ggp", bufs=1))
        hp = ctx.enter_context(tc.tile_pool(name="hp", bufs=1))
        ystp = ctx.enter_context(tc.tile_pool(name="ystp", bufs=1))
        ycp = ctx.enter_context(tc.tile_pool(name="ycp", bufs=2))
        cbp = ctx.enter_context(tc.tile_pool(name="cbp", bufs=2))
        tpl = ctx.enter_context(tc.tile_pool(name="tpl", bufs=2))
        clp = ctx.enter_context(tc.tile_pool(name="clp", bufs=2))
        pgp = ctx.enter_context(tc.tile_pool(name="pgp", bufs=1,
                                             space="PSUM"))
        pms = ctx.enter_context(tc.tile_pool(name="pms", bufs=1,
                                             space="PSUM"))
        p1 = ctx.enter_context(tc.tile_pool(name="p1", bufs=2, space="PSUM"))
        p2 = ctx.enter_context(tc.tile_pool(name="p2", bufs=2, space="PSUM"))

        # ---- constants (loaded once; weights resident) ----
        gwsb = cpool.tile([P, CK * E], f32)
        nc.sync.dma_start(gwsb[:].rearrange("p (k e) -> p k e", e=E),
                          gw.rearrange("(k p) e -> p k e", p=P))
        id8sb = cpool.tile([E, E], f32)
        nc.sync.dma_start(id8sb[:], id8)
        id128sb = cpool.tile([P, P], f32)
        nc.sync.dma_start(id128sb[:], id128)
        gbsb = cpool.tile([E, 1], f32)
        nc.sync.dma_start(gbsb[:], gb)
        b2sb = cpool.tile([E, C], f32)
        nc.sync.dma_start(b2sb[:], b2e)
        emsb = cpool.tile([16, E], f32)
        nc.sync.dma_start(emsb[:], emask)
        emb = cpool.tile([16, E, GW], f32)
        nc.vector.tensor_scalar_add(emb[:],
                                    emsb[:].to_broadcast([16, E, GW]), 0.0)
        b1sb = cpool.tile([P, HK], f32)
        nc.scalar.dma_start(b1sb[:].rearrange("p (k o) -> p k o", o=1),
                            b1.rearrange("(k p) one -> p k one", p=P))
        w1t = []
        for hk in range(HK):
            wt = cpool.tile([P, CK * P], bf16, tag=f"w1_{hk}")
            nc.sync.dma_start(wt[:], w1[hk])
            w1t.append(wt)
        w2t = []
        for hk in range(HK):
            wt = cpool.tile([P, C], bf16, tag=f"w2_{hk}")
            nc.sync.dma_start(wt[:], w2[hk])
            w2t.append(wt)

        iotaE_i = cpool.tile([P, TCH * E], dt.int32)
        nc.gpsimd.iota(iotaE_i[:], pattern=[[0, TCH], [1, E]], base=0,
                       channel_multiplier=0)
        iotaE = cpool.tile([P, TCH * E], f32)
        nc.vector.tensor_copy(iotaE[:], iotaE_i[:])
        toks_i = cpool.tile([P, TCH], dt.int32)
        nc.gpsimd.iota(toks_i[:], pattern=[[P, TCH]], base=0,
                       channel_multiplier=1)
        toksf = cpool.tile([P, TCH], f32)
        nc.vector.tensor_copy(toksf[:], toks_i[:])
        NIO = max(S16, CAP16)
        slotio_i = cpool.tile([16, NIO], dt.int32)
        nc.gpsimd.iota(slotio_i[:], pattern=[[16, NIO]], base=0,
                       channel_multiplier=1)
        slotio = cpool.tile([16, NIO], f32)
        nc.vector.tensor_copy(slotio[:], slotio_i[:])
        io8_i = cpool.tile([16, E], dt.int32)
        nc.gpsimd.iota(io8_i[:], pattern=[[1, E]], base=0,
                       channel_multiplier=0)
        io8 = cpool.tile([16, E], f32)
        nc.vector.tensor_copy(io8[:], io8_i[:])
        ones16 = cpool.tile([P, 16], f32)
        nc.vector.memset(ones16[:], 1.0)
        o16s = cpool.tile([16, 16], f32)
        nc.vector.memset(o16s[:], 1.0)
        zt = cpool.tile([P, 1024], bf16)
        nc.vector.memset(zt[:], 0.0)

        def stage_a(rep):
            """Home-side: gate + top-2 routing on own 1024 tokens; write
            the AllGather payload + b2 correction + per-expert counts."""
            par = rep % 2
            flatA = agin[par].rearrange("r w -> (r w)")

            # gate logits, expert-major (true fp32: fp32r flips top-2 picks)
            lgT = sa1.tile([E, ntok], f32, tag="big8")
            for nh in range(NH):
                lgps = pgp.tile([E, 512], f32, space="PSUM", tag="lgp")
                for k in range(CK):
                    xt = xtp.tile([P, 512], f32, tag="xt")
                    nc.sync.dma_start(
                        xt[:], xT[k * P:(k + 1) * P,
                                  nh * 512:(nh + 1) * 512])
                    nc.tensor.matmul(lgps[:],
                                     lhsT=gwsb[:, k * E:(k + 1) * E],
                                     rhs=xt[:],
                                     start=(k == 0), stop=(k == CK - 1))
                nc.vector.tensor_scalar_add(lgT[:, nh * 512:(nh + 1) * 512],
                                            lgps[:], gbsb[:, :1])
            lg = sa1.tile([P, TCH, E], f32, tag="lg")
            for t in range(TCH):
                ps = pms.tile([P, E], f32, space="PSUM", tag="misc")
                nc.tensor.transpose(ps[:], lgT[:, t * P:(t + 1) * P],
                                    id8sb[:])
                nc.vector.tensor_copy(lg[:, t, :], ps[:])
            # softmax over experts
            mx = sa.tile([P, TCH], f32, tag="mx")
            nc.vector.tensor_reduce(mx[:], lg[:], axis=AX.X, op=ALU.max)
            xm = sa.tile([P, TCH, E], f32, tag="xm")
            nc.vector.tensor_tensor(out=xm[:], in0=lg[:],
                                    in1=mx[:].to_broadcast([P, TCH, E]),
                                    op=ALU.subtract)
            ex = sa.tile([P, TCH, E], f32, tag="ex")
            nc.scalar.activation(ex[:], xm[:], AF.Exp)
            sm = sa.tile([P, TCH], f32, tag="sm")
            nc.vector.tensor_reduce(sm[:], ex[:], axis=AX.X, op=ALU.add)
            rs = sa.tile([P, TCH], f32, tag="rs")
            nc.vector.reciprocal(rs[:], sm[:])
            probs = sa.tile([P, TCH, E], f32, tag="probs")
            nc.vector.tensor_tensor(out=probs[:], in0=ex[:],
                                    in1=rs[:].to_broadcast([P, TCH, E]),
                                    op=ALU.mult)
            # top-2 by logits
            mig = sa.tile([P, TCH, 8], dt.uint32, tag="mig")
            for t in range(TCH):
                mv = sa.tile([P, 8], f32, tag="mv")
                nc.vector.max(mv[:], lg[:, t, :])
                nc.vector.max_index(mig[:, t, :], mv[:], lg[:, t, :])
            migf = sa.tile([P, TCH, 8], f32, tag="migf")
            nc.vector.tensor_copy(migf[:], mig[:])

            A = []
            g = []
            for r in range(2):
                Ar = sa1.tile([P, TCH, E], f32, tag=f"A{r}")
                nc.vector.tensor_tensor(
                    out=Ar[:],
                    in0=migf[:, :, r:r + 1].to_broadcast([P, TCH, E]),
                    in1=iotaE[:].rearrange("p (t e) -> p t e", e=E),
                    op=ALU.is_equal)
                gr = gpool.tile([P, TCH], f32, tag=f"g{r}")
                tmp = sa.tile([P, TCH, E], f32, tag="gt")
                nc.vector.tensor_tensor(out=tmp[:], in0=probs[:], in1=Ar[:],
                                        op=ALU.mult)
                nc.vector.tensor_reduce(gr[:], tmp[:], axis=AX.X, op=ALU.add)
                A.append(Ar)
                g.append(gr)
            M = sa1.tile([P, TCH, E], f32, tag="M")
            nc.vector.tensor_tensor(out=M[:], in0=A[0][:], in1=A[1][:],
                                    op=ALU.add)

            # per-expert token counts on 16 partitions (junk-tail masking)
            Mre = sa.tile([P, E, TCH], f32, tag="Mre")
            nc.vector.tensor_copy(Mre[:], M[:].rearrange("p t e -> p e t"))
            cntp = pms.tile([16, E * TCH], f32, space="PSUM", tag="misc")
            nc.tensor.matmul(cntp[:], lhsT=ones16[:],
                             rhs=Mre[:].rearrange("p e t -> p (e t)"),
                             start=True, stop=True)
            cntet = sa.tile([16, E, TCH], f32, tag="cntet")
            nc.vector.tensor_copy(cntet[:],
                                  cntp[:].rearrange("p (e t) -> p e t", e=E))
            cnt16 = gpool.tile([16, E], f32, tag="cnt16")
            nc.vector.tensor_reduce(cnt16[:], cntet[:], axis=AX.X, op=ALU.add)

            # candidate encoding: candU = t + 1 + RBIT*r (else -1)
            tokp1 = sa.tile([P, TCH], f32, tag="tokp1")
            nc.vector.tensor_scalar_add(tokp1[:], toksf[:], 2.0)
            base = sa.tile([P, TCH, E], f32, tag="base")
            nc.vector.scalar_tensor_tensor(
                out=base[:], in0=A[1][:], scalar=RBIT,
                in1=tokp1[:].to_broadcast([P, TCH, E]),
                op0=ALU.mult, op1=ALU.add)
            candU = sa1.tile([P, TCH, E], f32, tag="candU")
            nc.vector.tensor_tensor(out=candU[:], in0=base[:], in1=M[:],
                                    op=ALU.mult)
            nc.vector.tensor_scalar_add(candU[:], candU[:], -1.0)
            for ei in range(E):
                nc.scalar.dma_start(
                    flatA[OFF_U + ei * ntok:OFF_U + (ei + 1) * ntok]
                    .rearrange("(t p) -> p t", p=P),
                    candU[:, :, ei])

            # gate rows 2t+r of the payload
            for r in range(2):
                g64 = sa.tile([P, TCH, GW], f32, tag="g64")
                nc.vector.tensor_scalar_add(
                    g64[:], g[r][:].to_broadcast([P, TCH, GW]), 0.0)
                nc.scalar.dma_start(
                    agin[par, 0:AGG, :].rearrange(
                        "(tch p two) f -> p tch two f",
                        p=P, two=2)[:, :, r, :],
                    g64[:])

            # b2 correction sum_r g_r * b2[e_r] -> ycsta (DRAM staging)
            wtok = sa1.tile([P, TCH, E], f32, tag="wtok")
            nc.vector.tensor_tensor(
                out=wtok[:], in0=A[0][:],
                in1=g[0][:].to_broadcast([P, TCH, E]), op=ALU.mult)
            wtk1 = sa.tile([P, TCH, E], f32, tag="wtk1")
            nc.vector.tensor_tensor(
                out=wtk1[:], in0=A[1][:],
                in1=g[1][:].to_broadcast([P, TCH, E]), op=ALU.mult)
            nc.vector.tensor_tensor(out=wtok[:], in0=wtok[:], in1=wtk1[:],
                                    op=ALU.add)
            wTe = sa1.tile([E, TCH * P], f32, tag="big8")
            for t in range(TCH):
                pw = pms.tile([E, P], f32, space="PSUM", tag="misc")
                nc.tensor.transpose(pw[:], wtok[:, t, :], id128sb[:])
                nc.vector.tensor_copy(wTe[:, t * P:(t + 1) * P], pw[:])
            for t in range(TCH):
                yc = ycp.tile([P, C], bf16, tag="yc")
                for hh in range(2):
                    pc = pms.tile([P, 512], f32, space="PSUM", tag="misc")
                    nc.tensor.matmul(pc[:], lhsT=wTe[:, t * P:(t + 1) * P],
                                     rhs=b2sb[:, hh * 512:(hh + 1) * 512],
                                     start=True, stop=True)
                    nc.vector.tensor_copy(yc[:, hh * 512:(hh + 1) * 512],
                                          pc[:])
                nc.scalar.dma_start(
                    ycsta[par, t * P:(t + 1) * P, :], yc[:])
            return dict(cnt16=cnt16)

        def do_ag(rep):
            par = rep % 2
            nc.gpsimd.collective_compute(
                "AllGather", ALU.bypass, replica_groups=rg,
                ins=[agin[par]],
                outs=[agout[par].rearrange("n r w -> (n r) w")])

        def do_a2a(rep):
            par = rep % 2
            nc.gpsimd.collective_compute(
                "AllToAll", ALU.bypass, replica_groups=rg,
                ins=[a2in[par, 0:NCORE * CAP]
                     .rearrange("(n s) c -> n s c", n=NCORE)],
                outs=[a2out[par]])

        def stage_e_pre(rep):
            """Expert-side index build: own-expert global slot list,
            A2A row offsets, gather/scatter index triple."""
            par = rep % 2
            # zero the A2A input (scatter_add needs a clean base)
            a2flat = a2in[par].rearrange("s c -> (s c)")
            zchunk = P * 1024
            nz = NCORE * CAP * C // zchunk
            for z in range(nz):
                nc.sync.dma_start(
                    a2flat[z * zchunk:(z + 1) * zchunk]
                    .rearrange("(p f) -> p f", p=P),
                    zt[:])

            flatO = agout[par].rearrange("n r w -> (n r w)")
            ctU = mb.tile([16, NCORE * 512], f32, tag="ctU")
            cpart = mb.tile([16, NCORE], f32, tag="cpart")
            for j in range(NCORE):
                rU = mb.tile([16, 512], f32, tag="rU")
                nc.scalar.dma_start(
                    rU[:], flatO[j * AGR * GW + OFF_U:
                                 j * AGR * GW + OFF_U + E * ntok]
                    .rearrange("(f p) -> p f", p=16))
                # mask to own expert (others/pads -> -1), then rebase the
                # valid entries only: + j*ntok - 1 (u encodes t_local+1)
                uj = ctU[:, j * 512:(j + 1) * 512]
                nc.vector.scalar_tensor_tensor(
                    out=uj, in0=rU[:], scalar=1.0,
                    in1=emb[:].rearrange("p e w -> p (e w)"),
                    op0=ALU.add, op1=ALU.mult)
                nc.vector.tensor_scalar_add(uj, uj, -1.0)
                vm = mb.tile([16, 512], f32, tag="vm")
                nc.vector.tensor_scalar(vm[:], uj, 0.0, 0.0,
                                        op0=ALU.is_ge, op1=ALU.add)
                nc.vector.scalar_tensor_tensor(
                    out=uj, in0=vm[:], scalar=float(j * ntok - 1),
                    in1=uj, op0=ALU.mult, op1=ALU.add)
                nc.vector.tensor_reduce(cpart[:, j:j + 1], vm[:],
                                        axis=AX.X, op=ALU.add)

            # per-home counts -> A2A row offsets off[j] = j*CAP - start_j
            chp = pms.tile([16, NCORE], f32, space="PSUM", tag="misc")
            nc.tensor.matmul(chp[:], lhsT=o16s[:], rhs=cpart[:],
                             start=True, stop=True)
            ch = mb.tile([16, NCORE], f32, tag="ch")
            nc.vector.tensor_copy(ch[:], chp[:])

            # two-level compaction (sparse_gather input free dim is capped
            # at 512 on HW): per-home 512 -> CAP16 (junk tail -> -1 via the
            # home count), then one gather over the padded concat.
            r2in = mb.tile([16, NCORE * CAP16], f32, tag="r2in")
            for j in range(NCORE):
                r1 = mb.tile([16, CAP16], f32, tag="r1")
                nc.vector.memset(r1[:], -1.0)
                nfd1 = mb.tile([1, 1], dt.uint32, tag="nfd1")
                nc.gpsimd.sparse_gather(
                    r1[:], ctU[:, j * 512:(j + 1) * 512],
                    num_found=nfd1[:])
                mj = mb.tile([16, CAP16], f32, tag="mj")
                nc.vector.tensor_tensor(
                    out=mj[:], in0=slotio[:, :CAP16],
                    in1=ch[:, j:j + 1].to_broadcast([16, CAP16]),
                    op=ALU.is_lt)
                nc.vector.tensor_scalar_add(r1[:], r1[:], 1.0)
                nc.vector.tensor_tensor(out=r1[:], in0=r1[:], in1=mj[:],
                                        op=ALU.mult)
                nc.vector.tensor_scalar_add(
                    r2in[:, j * CAP16:(j + 1) * CAP16], r1[:], -1.0)
            tkU = mb.tile([16, S16], f32, tag="tkU")
            nc.vector.memset(tkU[:], -1.0)
            nfd = mb.tile([1, 1], dt.uint32, tag="nfd")
            nc.gpsimd.sparse_gather(tkU[:], r2in[:], num_found=nfd[:])
            pf = ch
            for sh in (1, 2, 4):
                nxtp = mb.tile([16, NCORE], f32, tag=f"pf{sh}")
                nc.vector.tensor_copy(nxtp[:, 0:sh], pf[:, 0:sh])
                nc.vector.tensor_tensor(out=nxtp[:, sh:NCORE],
                                        in0=pf[:, sh:NCORE],
                                        in1=pf[:, 0:NCORE - sh], op=ALU.add)
                pf = nxtp
            excl = mb.tile([16, NCORE], f32, tag="excl")
            nc.vector.tensor_tensor(out=excl[:], in0=pf[:], in1=ch[:],
                                    op=ALU.subtract)
            off = mb.tile([16, NCORE], f32, tag="off")
            nc.vector.scalar_tensor_tensor(
                out=off[:], in0=io8[:], scalar=float(CAP), in1=excl[:],
                op0=ALU.mult, op1=ALU.subtract)
            total = pf[:, NCORE - 1:NCORE]

            # decode: rank bit, global token id (clamped), home, gate row
            rb = mb.tile([16, S16], f32, tag="rb")
            nc.vector.tensor_scalar(rb[:], tkU[:], RBIT, 0.0,
                                    op0=ALU.is_ge, op1=ALU.add)
            xf = mb.tile([16, S16], f32, tag="xf")
            nc.vector.scalar_tensor_tensor(
                out=xf[:], in0=rb[:], scalar=-RBIT, in1=tkU[:],
                op0=ALU.mult, op1=ALU.add)
            nc.vector.tensor_scalar(xf[:], xf[:], 0.0, float(NTOT - 1),
                                    op0=ALU.max, op1=ALU.min)
            js = mb.tile([16, S16], f32, tag="js")
            nc.vector.tensor_scalar(js[:], xf[:], float(ntok), 0.0,
                                    op0=ALU.is_ge, op1=ALU.add)
            for k in range(2, NCORE):
                jtmp = mb.tile([16, S16], f32, tag="jtmp")
                nc.vector.tensor_scalar(jtmp[:], xf[:], float(k * ntok),
                                        0.0, op0=ALU.is_ge, op1=ALU.add)
                nc.vector.tensor_tensor(out=js[:], in0=js[:], in1=jtmp[:],
                                        op=ALU.add)
            # gate row = 2*t_g + r + (AGR-2048)*home
            grow = mb.tile([16, S16], f32, tag="grow")
            nc.vector.scalar_tensor_tensor(
                out=grow[:], in0=xf[:], scalar=2.0, in1=rb[:],
                op0=ALU.mult, op1=ALU.add)
            gtmp = mb.tile([16, S16], f32, tag="gtmp")
            nc.vector.scalar_tensor_tensor(
                out=gtmp[:], in0=js[:], scalar=float(AGR - AGG), in1=grow[:],
                op0=ALU.mult, op1=ALU.add)
            # scatter row = slot + off[home]; pads (slot >= total) -> -1
            osel = mb.tile([16, S16], f32, tag="osel")
            nc.vector.memset(osel[:], 0.0)
            for k in range(NCORE):
                ieq = mb.tile([16, S16], f32, tag="ieq")
                nc.vector.tensor_scalar(ieq[:], js[:], float(k), 0.0,
                                        op0=ALU.is_equal, op1=ALU.add)
                nc.vector.tensor_tensor(
                    out=ieq[:], in0=ieq[:],
                    in1=off[:, k:k + 1].to_broadcast([16, S16]),
                    op=ALU.mult)
                nc.vector.tensor_tensor(out=osel[:], in0=osel[:],
                                        in1=ieq[:], op=ALU.add)
            scat = mb.tile([16, S16], f32, tag="scat")
            nc.vector.tensor_tensor(out=scat[:], in0=slotio[:, :S16],
                                    in1=osel[:], op=ALU.add)
            # junk tail -> trash row NCORE*CAP (payload junk collides only
            # in trash; skipped/negative indices hang the SWDGE scatter)
            msk = mb.tile([16, S16], f32, tag="msk")
            nc.vector.tensor_tensor(out=msk[:], in0=slotio[:, :S16],
                                    in1=total.to_broadcast([16, S16]),
                                    op=ALU.is_lt)
            nc.vector.tensor_scalar_add(scat[:], scat[:],
                                        float(-NCORE * CAP))
            nc.vector.tensor_tensor(out=scat[:], in0=scat[:], in1=msk[:],
                                    op=ALU.mult)
            nc.vector.tensor_scalar_add(scat[:], scat[:],
                                        float(NCORE * CAP))

            if debug and rep == 0:
                nc.scalar.dma_start(dbgA[0], tkU[:])
                nc.scalar.dma_start(dbgA[1], xf[:])
                nc.scalar.dma_start(dbgA[2], scat[:])
                nc.scalar.dma_start(dbgA[3], gtmp[:])
                nc.scalar.dma_start(dbgC[0], ch[:])
                nc.scalar.dma_start(dbgC[1], off[:])
                nc.scalar.dma_start(dbgC[2], cpart[:])
            trip = tpl.tile([P, 3 * S16], dt.int16, tag="trip")
            nc.vector.tensor_copy(trip[0:16, 0:S16], xf[:])
            nc.vector.tensor_copy(trip[0:16, S16:2 * S16], scat[:])
            nc.vector.tensor_copy(trip[0:16, 2 * S16:3 * S16], gtmp[:])
            for sz in (16, 32, 64):
                nc.scalar.dma_start(trip[sz:2 * sz, :], trip[0:sz, :])
            return trip

        def stage_e_main(rep, trip):
            """Expert MLP from resident weights; scatter to A2A input."""
            par = rep % 2
            t16X = trip[:, 0:S16]
            t16S = trip[:, S16:2 * S16]
            t16G = trip[:, 2 * S16:3 * S16]
            for (hoff, W) in HALVES:
                h16 = hoff // 16
                ggt = ggp.tile([P, W // 128, GW], f32, tag="gg")
                nc.gpsimd.dma_gather(
                    ggt[:], agout[par].rearrange("n r w -> (n r) w"),
                    t16G[:, h16:h16 + W // 16], W, W, GW)
                if level < 4:
                    continue
                xg = xgp.tile([P, CK, W], bf16, tag="xg")
                nc.gpsimd.dma_gather(xg[:], xtm,
                                     t16X[:, h16:h16 + W // 16],
                                     W, W, C, transpose=True)
                if level < 5:
                    continue
                hs = []
                for hk in range(HK):
                    ps = p1.tile([P, W], f32, space="PSUM", tag="ps1")
                    for k in range(CK):
                        nc.tensor.matmul(
                            ps[:], lhsT=w1t[hk][:, k * P:(k + 1) * P],
                            rhs=xg[:, k, :],
                            start=(k == 0), stop=(k == CK - 1))
                    ht = hp.tile([P, W], bf16, tag=f"h{hk}")
                    nc.scalar.activation(ht[:], ps[:], AF.Relu,
                                         bias=b1sb[:, hk:hk + 1])
                    hs.append(ht)
                if level < 6:
                    continue
                yst = ystp.tile([P, W // 128, C], bf16, tag="yst")
                for sub in range(W // 128):
                    lo = sub * 128
                    psA = p2.tile([P, 512], f32, space="PSUM",
                                  tag="ps2a")
                    psB = p2.tile([P, 512], f32, space="PSUM",
                                  tag="ps2b")
                    for hk in range(HK):
                        nc.tensor.matmul(
                            psA[:], lhsT=hs[hk][:, lo:lo + 128],
                            rhs=w2t[hk][:, 0:512],
                            start=(hk == 0), stop=(hk == HK - 1))
                        nc.tensor.matmul(
                            psB[:], lhsT=hs[hk][:, lo:lo + 128],
                            rhs=w2t[hk][:, 512:1024],
                            start=(hk == 0), stop=(hk == HK - 1))
                    nc.vector.tensor_tensor(
                        out=yst[:, sub, 0:512], in0=psA[:],
                        in1=ggt[:, sub, 0:1].to_broadcast([P, 512]),
                        op=ALU.mult)
                    nc.vector.tensor_tensor(
                        out=yst[:, sub, 512:1024], in0=psB[:],
                        in1=ggt[:, sub, 0:1].to_broadcast([P, 512]),
                        op=ALU.mult)
                if level < 7:
                    continue
                nc.gpsimd.dma_scatter_add(
                    a2in[par], yst[:], t16S[:, h16:h16 + W // 16],
                    W, W, C, queue_num=1)

        def stage_c_pre(rep, actx):
            """Home-side combine lists (own tokens per expert)."""
            par = rep % 2
            cnt16 = actx["cnt16"]
            flatA = agin[par].rearrange("r w -> (r w)")
            t16s = []
            for e2 in range(NCORE):
                cw = cbp.tile([16, GW], f32, tag="cw")
                nc.scalar.dma_start(
                    cw[:], flatA[OFF_U + e2 * ntok:OFF_U + (e2 + 1) * ntok]
                    .rearrange("(f p) -> p f", p=16))
                msk = cbp.tile([16, CAP16], f32, tag="cmsk")
                nc.vector.tensor_tensor(
                    out=msk[:], in0=slotio[:, :CAP16],
                    in1=cnt16[:, e2:e2 + 1].to_broadcast([16, CAP16]),
                    op=ALU.is_lt)
                tk = cbp.tile([16, CAP16], f32, tag="ctk")
                nc.vector.memset(tk[:], -1.0)
                nfd = cbp.tile([1, 1], dt.uint32, tag="cnfd")
                nc.gpsimd.sparse_gather(tk[:, :CAP16], cw[:],
                                        num_found=nfd[:])
                # decode t+1 out of candU = t + 1 + RBIT*r
                rb = cbp.tile([16, CAP16], f32, tag="crb")
                nc.vector.tensor_scalar(rb[:], tk[:], RBIT, 0.0,
                                        op0=ALU.is_ge, op1=ALU.add)
                xff = cbp.tile([16, CAP16], f32, tag="cxff")
                nc.vector.scalar_tensor_tensor(
                    out=xff[:], in0=rb[:], scalar=-RBIT, in1=tk[:],
                    op0=ALU.mult, op1=ALU.add)
                nc.vector.tensor_scalar(xff[:], xff[:], 1.0, float(ntok),
                                        op0=ALU.max, op1=ALU.min)
                nc.vector.tensor_scalar_add(xff[:], xff[:], -1.0)
                stf = cbp.tile([16, CAP16], f32, tag="cstf")
                nc.vector.scalar_tensor_tensor(
                    out=stf[:], in0=xff[:], scalar=float(-ntok),
                    in1=msk[:], op0=ALU.add, op1=ALU.mult)
                nc.vector.tensor_scalar_add(stf[:], stf[:], float(ntok))
                if debug and rep == 0:
                    nc.scalar.dma_start(dbgH[e2], stf[:])
                t16 = clp.tile([P, CAP16], dt.int16, tag=f"ct16_{e2}")
                nc.vector.tensor_copy(t16[0:16, :], stf[:])
                for sz in (16, 32, 64):
                    nc.scalar.dma_start(t16[sz:2 * sz, :], t16[0:sz, :])
                t16s.append(t16)
            return t16s

        def stage_c_main(rep, t16s):
            """yT = b2 correction + scatter-add of returned rows."""
            par = rep % 2
            nc.sync.dma_start(yT[0:ntok, :], ycsta[par])
            for e2 in range(NCORE):
                bt = cbp.tile([P, CAP // 128, C], bf16, tag="bt")
                nc.sync.dma_start(
                    bt[:], a2out[par, e2].rearrange("(s p) c -> p s c",
                                                    p=P))
                nc.gpsimd.dma_scatter_add(yT, bt[:], t16s[e2][:], CAP, CAP,
                                          C, queue_num=1)

        # software pipeline: E_pre(r)/C_pre(r) first (need only AG(r)/
        # A(r)), then A(r+1)+AG(r+1), then the expert MLP of r
        actxs = {0: stage_a(0)}
        if level >= 1:
            do_ag(0)
        for rep in range(repeat):
            trip = stage_e_pre(rep) if level >= 2 else None
            cl = (stage_c_pre(rep, actxs.pop(rep))
                  if level >= 9 else None)
            if rep + 1 < repeat:
                actxs[rep + 1] = stage_a(rep + 1)
                if level >= 1:
                    do_ag(rep + 1)
            if level >= 3:
                stage_e_main(rep, trip)
            if level >= 8:
                do_a2a(rep)
            if level >= 9:
                stage_c_main(rep, cl)

    return nc


# ---------------- host side ----------------

def _host_route(xf, gate_w, gate_b):
    logits = xf.astype(np.float32) @ gate_w.astype(np.float32) + gate_b
    return np.argpartition(-logits, TOPK - 1, axis=1)[:, :TOPK]


def _host_caps(order, ntok=NTOK):
    cnt = np.bincount(order.ravel(), minlength=E)
    slot = int(np.ceil((cnt.max() + 16) / 128.0) * 128)
    ncore = order.shape[0] // ntok
    pair = np.zeros((E, ncore), np.int64)
    for j in range(ncore):
        sl = order[j * ntok:(j + 1) * ntok]
        pair[:, j] = np.bincount(sl.ravel(), minlength=E)
    cap = int(np.ceil((pair.max() + 16) / 128.0) * 128)
    assert slot // 16 <= 512, f"slot overflow: {slot}"
    return slot, cap


def kernel(x, gate_w, gate_b, w1, b1, w2, b2):
    from concourse.bass_utils import run_bass_kernel_spmd
    import ml_dtypes

    x = np.asarray(x, np.float32)
    gate_w = np.asarray(gate_w, np.float32)
    gate_b = np.asarray(gate_b, np.float32)
    w1 = np.asarray(w1, np.float32)
    b1 = np.asarray(b1, np.float32)
    w2 = np.asarray(w2, np.float32)
    b2 = np.asarray(b2, np.float32)

    # w1 in lhsT-chunk layout: [E, HK, P(c in chunk), CK*P(h)]
    w1r = np.ascontiguousarray(
        (w1.reshape(E, CK, P, HK, P).transpose(0, 3, 2, 1, 4)
         .reshape(E, HK, P, C)).astype(ml_dtypes.bfloat16))
    w2b = np.ascontiguousarray(
        w2.reshape(E, HK, P, C).astype(ml_dtypes.bfloat16))

    b, t, c = x.shape
    xf = x.reshape(b * t, c)
    order = _host_route(xf, gate_w, gate_b)
    S = _host_caps(order)
    nc = build_program(S)

    xtm_full = np.ascontiguousarray(xf.astype(ml_dtypes.bfloat16))
    shared = {
        "xtm": xtm_full,
        "gw": gate_w,
        "gb": gate_b.reshape(E, 1).copy(),
        "b2e": b2,
        "id8": np.eye(E, dtype=np.float32),
        "id128": np.eye(P, dtype=np.float32),
    }
    in_maps = []
    for cc in range(NCORE):
        sl = xf[cc * NTOK:(cc + 1) * NTOK]
        m = dict(shared)
        m["xT"] = np.ascontiguousarray(sl.T)
        m["w1"] = w1r[cc]
        m["b1"] = np.ascontiguousarray(b1[cc].reshape(H, 1))
        m["w2"] = w2b[cc]
        em = np.zeros((16, E), np.float32)
        em[:, cc] = 1.0
        m["emask"] = em
        in_maps.append(m)

    global LAST_BUILD, LAST_S
    LAST_BUILD = (nc, in_maps)
    LAST_S = S
    res = run_bass_kernel_spmd(nc, in_maps, core_ids=list(range(NCORE)))
    outs = [np.asarray(r["yT"][:NTOK]).astype(np.float32)
            for r in res.results]
    y = np.concatenate(outs, axis=0).reshape(b, t, c)
    return y


# revision 30
# speedup vs baseline: 5.3647x; 2.5866x over previous
"""Trainium2 Bass kernel for top-2 MoE (nn_ExpertMemory) — expert parallel.

Model (reference semantics):
    logits = x @ gate_w + gate_b          # (N, E)
    probs  = softmax(logits)
    gates, idx = top_k(probs, 2)
    out[n] = sum_k gates[n,k] * (relu(x[n] @ w1[e] + b1[e]) @ w2[e] + b2[e]),
             e = idx[n,k]

Sharding: expert parallelism. Core c owns expert c; its w1/w2 (8 MB bf16)
stay RESIDENT in SBUF across iterations, eliminating the 64 MB/core
weight streaming of the data-parallel layout. Tokens are data-parallel
for routing only: core j ("home" of tokens [1024j, 1024j+1024)) computes
the fp32 gate + top-2 for its tokens and AllGathers a compact payload:
2048 gate rows (row 2t+r, 64-wide) + 128 candidate rows encoding
candU = t + 1 + 16384*r for each (expert, token) pick (else -1).
Each expert core then:
  1. reads all 8 homes' candidate arrays (strided [16,512] views), masks
     to its own expert via a one-hot input (emask), rebases to global
     token ids, and runs ONE sparse_gather over the [16, 4096] concat ->
     tight-packed global slot list; home runs stay contiguous and in
     home-local order because sparse_gather traverses f-major (f*16+p)
  2. dma_gather(transpose) of token rows from the REPLICATED full x
     (xtm[8192, C] bf16 on every core; dispatch reads are local) and of
     gate rows from the AllGather output (row 2*t_g + r + 128*home)
  3. 2-layer MLP from SBUF-resident weights, in two slot halves; gate
     applied on the PSUM->SBUF copy
  4. dma_scatter_add of rows into a zeroed AllToAll input at row
     home*CAP + pos-within-home-run (pads get index -1: trailing
     negatives are skipped by the scatter)
  5. AllToAll returns each home its tokens' expert rows; the home
     scatter-adds region e rows into yT (pre-initialized with the
     gate-weighted b2 correction) using its own per-expert token lists,
     whose order matches the expert core's runs by construction.
All staging is double-buffered by repeat parity; stage A of rep i+1 and
its AllGather are emitted before stage E of rep i so routing and
collectives overlap the expert MLP.
"""

import numpy as np
from contextlib import ExitStack

import concourse.bass as bass
import concourse.tile as tile
import concourse.mybir as mybir
from concourse import bacc

dt = mybir.dt
AF = mybir.ActivationFunctionType
ALU = mybir.AluOpType
AX = mybir.AxisListType

P = 128

# problem constants
B, T, C, E, H, TOPK = 4, 2048, 1024, 8, 2048, 2
NCORE = 8
NTOK = B * T // NCORE   # tokens per home core (1024)
NTOT = B * T            # all tokens (8192)
TCH = NTOK // P         # token chunks per home (8)
CK = C // P             # C chunks (8)
HK = H // P             # H chunks (16)
GW = 64                 # gather element width (64 f32 = 256 B minimum)
GR = 64                 # gate row width (64 f32 = 256 B, gather minimum)
NH = NTOK // 512        # 512-wide token halves for the gate matmul
RBIT = 16384.0          # rank-bit offset in candU encoding
AGG = 2 * NTOK          # gate rows in AG payload (2048)
OFF_U = AGG * GR        # f32 offset of the candidate region
NWORDS = AGG * GR + E * NTOK  # AG payload f32 words per rank (40960)
GROWS = NWORDS // GR    # payload as 64B gate rows (2560)


def _tiles(s):
    out = []
    off = 0
    rem = s
    while rem > 512:
        out.append((off, 512))
        off += 512
        rem -= 512
    if rem:
        out.append((off, rem))
    return out


def build_program(S, ntok=NTOK, level=9, repeat=1, debug=False):
    nc = _build(S, ntok=ntok, level=level, repeat=repeat, debug=debug)
    nc.compile()
    return nc


def _build(S, ntok=NTOK, level=9, repeat=1, debug=False):
    """S: (SLOT, CAP): SLOT = global per-expert slot capacity (multiple
    of 128, SLOT/16 <= 512); CAP = per-(expert, home) A2A region rows
    (multiple of 128)."""
    SLOT, CAP = int(S[0]), int(S[1])
    assert SLOT % 128 == 0 and SLOT // 16 <= 512
    assert CAP % 128 == 0
    S16 = SLOT // 16
    CAP16 = CAP // 16
    # slot chunks of <= 512 (transpose dma_gather num_idxs HW cap), each a
    # multiple of 128
    HALVES = _tiles(SLOT)

    nc = bacc.Bacc("TRN2", target_bir_lowering=False, debug=False,
                   num_swdge_queues=2)

    f32, bf16 = dt.float32, dt.bfloat16
    xT = nc.dram_tensor("xT", [C, ntok], f32, kind="ExternalInput").ap()
    xtm = nc.dram_tensor("xtm", [NTOT, C], bf16, kind="ExternalInput").ap()
    gw = nc.dram_tensor("gw", [C, E], f32, kind="ExternalInput").ap()
    gb = nc.dram_tensor("gb", [E, 1], f32, kind="ExternalInput").ap()
    w1 = nc.dram_tensor("w1", [HK, P, CK * P], bf16,
                        kind="ExternalInput").ap()
    b1 = nc.dram_tensor("b1", [H, 1], f32, kind="ExternalInput").ap()
    w2 = nc.dram_tensor("w2", [HK, P, C], bf16, kind="ExternalInput").ap()
    b2e = nc.dram_tensor("b2e", [E, C], f32, kind="ExternalInput").ap()
    emask = nc.dram_tensor("emask", [16, 512], f32,
                           kind="ExternalInput").ap()
    id8 = nc.dram_tensor("id8", [E, E], f32, kind="ExternalInput").ap()
    id128 = nc.dram_tensor("id128", [P, P], f32, kind="ExternalInput").ap()
    yT = nc.dram_tensor("yT", [ntok + 16, C], bf16, kind="ExternalOutput").ap()

    # staging, double-buffered by repeat parity
    agin = nc.dram_tensor("agin", [2, NWORDS], f32).ap()
    agout = nc.dram_tensor("agout", [2, NCORE, NWORDS], f32).ap()
    a2in = nc.dram_tensor("a2in", [2, NCORE * CAP + 16, C], bf16).ap()
    a2out = nc.dram_tensor("a2out", [2, NCORE, CAP, C], bf16).ap()
    ycsta = nc.dram_tensor("ycsta", [2, ntok, C], bf16).ap()
    if debug:
        S16d = int(S[0]) // 16
        dbgA = nc.dram_tensor("dbgA", [4, 16, S16d], f32,
                              kind="ExternalOutput").ap()
        dbgC = nc.dram_tensor("dbgC", [3, 16, NCORE], f32,
                              kind="ExternalOutput").ap()
        dbgH = nc.dram_tensor("dbgH", [NCORE, 16, int(S[1]) // 16], f32,
                              kind="ExternalOutput").ap()
    rg = [list(range(NCORE))]

    with tile.TileContext(nc) as tc, ExitStack() as ctx:
        cpool = ctx.enter_context(tc.tile_pool(name="const", bufs=1))
        gpool = ctx.enter_context(tc.tile_pool(name="gk", bufs=2))
        sa = ctx.enter_context(tc.tile_pool(name="sa", bufs=2))
        sa1 = ctx.enter_context(tc.tile_pool(name="sa1", bufs=2))
        xtp = ctx.enter_context(tc.tile_pool(name="xt", bufs=2))
        mb = ctx.enter_context(tc.tile_pool(name="mb", bufs=1))
        xgp = ctx.enter_context(tc.tile_pool(name="xgp", bufs=1))
        ggp = ctx.enter_context(tc.tile_pool(name="ggp", bufs=1))
        hp = ctx.enter_context(tc.tile_pool(name="hp", bufs=1))
        ystp = ctx.enter_context(tc.tile_pool(name="ystp", bufs=1))
        ycp = ctx.enter_context(tc.tile_pool(name="ycp", bufs=2))
        cbp = ctx.enter_context(tc.tile_pool(name="cbp", bufs=2))
        tpl = ctx.enter_context(tc.tile_pool(name="tpl", bufs=2))
        clp = ctx.enter_context(tc.tile_pool(name="clp", bufs=2))
        pgp = ctx.enter_context(tc.tile_pool(name="pgp", bufs=1,
                                             space="PSUM"))
        pms = ctx.enter_context(tc.tile_pool(name="pms", bufs=1,
                                             space="PSUM"))
        p1 = ctx.enter_context(tc.tile_pool(name="p1", bufs=2, space="PSUM"))
        p2 = ctx.enter_context(tc.tile_pool(name="p2", bufs=2, space="PSUM"))

        # ---- constants (loaded once; weights resident) ----
        gwsb = cpool.tile([P, CK * E], f32)
        nc.sync.dma_start(gwsb[:].rearrange("p (k e) -> p k e", e=E),
                          gw.rearrange("(k p) e -> p k e", p=P))
        id8sb = cpool.tile([E, E], f32)
        nc.sync.dma_start(id8sb[:], id8)
        id128sb = cpool.tile([P, P], f32)
        nc.sync.dma_start(id128sb[:], id128)
        gbsb = cpool.tile([E, 1], f32)
        nc.sync.dma_start(gbsb[:], gb)
        b2sb = cpool.tile([E, C], f32)
        nc.sync.dma_start(b2sb[:], b2e)
        emsb = cpool.tile([16, 512], f32)
        nc.sync.dma_start(emsb[:], emask)
        b1sb = cpool.tile([P, HK], f32)
        nc.scalar.dma_start(b1sb[:].rearrange("p (k o) -> p k o", o=1),
                            b1.rearrange("(k p) one -> p k one", p=P))
        w1t = []
        for hk in range(HK):
            wt = cpool.tile([P, CK * P], bf16, tag=f"w1_{hk}")
            nc.sync.dma_start(wt[:], w1[hk])
            w1t.append(wt)
        w2t = []
        for hk in range(HK):
            wt = cpool.tile([P, C], bf16, tag=f"w2_{hk}")
            nc.sync.dma_start(wt[:], w2[hk])
            w2t.append(wt)

        iotaE_i = cpool.tile([P, TCH * E], dt.int32)
        nc.gpsimd.iota(iotaE_i[:], pattern=[[0, TCH], [1, E]], base=0,
                       channel_multiplier=0)
        iotaE = cpool.tile([P, TCH * E], f32)
        nc.vector.tensor_copy(iotaE[:], iotaE_i[:])
        toks_i = cpool.tile([P, TCH], dt.int32)
        nc.gpsimd.iota(toks_i[:], pattern=[[P, TCH]], base=0,
                       channel_multiplier=1)
        toksf = cpool.tile([P, TCH], f32)
        nc.vector.tensor_copy(toksf[:], toks_i[:])
        NIO = max(S16, CAP16)
        slotio_i = cpool.tile([16, NIO], dt.int32)
        nc.gpsimd.iota(slotio_i[:], pattern=[[16, NIO]], base=0,
                       channel_multiplier=1)
        slotio = cpool.tile([16, NIO], f32)
        nc.vector.tensor_copy(slotio[:], slotio_i[:])
        io8_i = cpool.tile([16, E], dt.int32)
        nc.gpsimd.iota(io8_i[:], pattern=[[1, E]], base=0,
                       channel_multiplier=0)
        io8 = cpool.tile([16, E], f32)
        nc.vector.tensor_copy(io8[:], io8_i[:])
        ones16 = cpool.tile([P, 16], f32)
        nc.vector.memset(ones16[:], 1.0)
        o16s = cpool.tile([16, 16], f32)
        nc.vector.memset(o16s[:], 1.0)
        zt = cpool.tile([P, 1024], bf16)
        nc.vector.memset(zt[:], 0.0)

        def stage_a(rep):
            """Home-side: gate + top-2 routing on own 1024 tokens; write
            the AllGather payload + b2 correction + per-expert counts."""
            par = rep % 2
            flatA = agin[par]

            # gate logits, expert-major (true fp32: fp32r flips top-2 picks)
            lgT = sa1.tile([E, ntok], f32, tag="big8")
            for nh in range(NH):
                lgps = pgp.tile([E, 512], f32, space="PSUM", tag="lgp")
                for k in range(CK):
                    xt = xtp.tile([P, 512], f32, tag="xt")
                    nc.sync.dma_start(
                        xt[:], xT[k * P:(k + 1) * P,
                                  nh * 512:(nh + 1) * 512])
                    nc.tensor.matmul(lgps[:],
                                     lhsT=gwsb[:, k * E:(k + 1) * E],
                                     rhs=xt[:],
                                     start=(k == 0), stop=(k == CK - 1))
                nc.vector.tensor_scalar_add(lgT[:, nh * 512:(nh + 1) * 512],
                                            lgps[:], gbsb[:, :1])
            lg = sa1.tile([P, TCH, E], f32, tag="lg")
            for t in range(TCH):
                ps = pms.tile([P, E], f32, space="PSUM", tag="misc")
                nc.tensor.transpose(ps[:], lgT[:, t * P:(t + 1) * P],
                                    id8sb[:])
                nc.vector.tensor_copy(lg[:, t, :], ps[:])
            # softmax over experts
            mx = sa.tile([P, TCH], f32, tag="mx")
            nc.vector.tensor_reduce(mx[:], lg[:], axis=AX.X, op=ALU.max)
            xm = sa.tile([P, TCH, E], f32, tag="xm")
            nc.vector.tensor_tensor(out=xm[:], in0=lg[:],
                                    in1=mx[:].to_broadcast([P, TCH, E]),
                                    op=ALU.subtract)
            ex = sa.tile([P, TCH, E], f32, tag="ex")
            nc.scalar.activation(ex[:], xm[:], AF.Exp)
            sm = sa.tile([P, TCH], f32, tag="sm")
            nc.vector.tensor_reduce(sm[:], ex[:], axis=AX.X, op=ALU.add)
            rs = sa.tile([P, TCH], f32, tag="rs")
            nc.vector.reciprocal(rs[:], sm[:])
            probs = sa.tile([P, TCH, E], f32, tag="probs")
            nc.vector.tensor_tensor(out=probs[:], in0=ex[:],
                                    in1=rs[:].to_broadcast([P, TCH, E]),
                                    op=ALU.mult)
            # top-2 by logits
            mig = sa.tile([P, TCH, 8], dt.uint32, tag="mig")
            for t in range(TCH):
                mv = sa.tile([P, 8], f32, tag="mv")
                nc.vector.max(mv[:], lg[:, t, :])
                nc.vector.max_index(mig[:, t, :], mv[:], lg[:, t, :])
            migf = sa.tile([P, TCH, 8], f32, tag="migf")
            nc.vector.tensor_copy(migf[:], mig[:])

            A = []
            g = []
            for r in range(2):
                Ar = sa1.tile([P, TCH, E], f32, tag=f"A{r}")
                nc.vector.tensor_tensor(
                    out=Ar[:],
                    in0=migf[:, :, r:r + 1].to_broadcast([P, TCH, E]),
                    in1=iotaE[:].rearrange("p (t e) -> p t e", e=E),
                    op=ALU.is_equal)
                gr = gpool.tile([P, TCH], f32, tag=f"g{r}")
                tmp = sa.tile([P, TCH, E], f32, tag="gt")
                nc.vector.tensor_tensor(out=tmp[:], in0=probs[:], in1=Ar[:],
                                        op=ALU.mult)
                nc.vector.tensor_reduce(gr[:], tmp[:], axis=AX.X, op=ALU.add)
                A.append(Ar)
                g.append(gr)
            M = sa1.tile([P, TCH, E], f32, tag="M")
            nc.vector.tensor_tensor(out=M[:], in0=A[0][:], in1=A[1][:],
                                    op=ALU.add)

            # per-expert token counts on 16 partitions (junk-tail masking)
            Mre = sa.tile([P, E, TCH], f32, tag="Mre")
            nc.vector.tensor_copy(Mre[:], M[:].rearrange("p t e -> p e t"))
            cntp = pms.tile([16, E * TCH], f32, space="PSUM", tag="misc")
            nc.tensor.matmul(cntp[:], lhsT=ones16[:],
                             rhs=Mre[:].rearrange("p e t -> p (e t)"),
                             start=True, stop=True)
            cntet = sa.tile([16, E, TCH], f32, tag="cntet")
            nc.vector.tensor_copy(cntet[:],
                                  cntp[:].rearrange("p (e t) -> p e t", e=E))
            cnt16 = gpool.tile([16, E], f32, tag="cnt16")
            nc.vector.tensor_reduce(cnt16[:], cntet[:], axis=AX.X, op=ALU.add)

            # candidate encoding: candU = t + 1 + RBIT*r (else -1)
            tokp1 = sa.tile([P, TCH], f32, tag="tokp1")
            nc.vector.tensor_scalar_add(tokp1[:], toksf[:], 2.0)
            base = sa.tile([P, TCH, E], f32, tag="base")
            nc.vector.scalar_tensor_tensor(
                out=base[:], in0=A[1][:], scalar=RBIT,
                in1=tokp1[:].to_broadcast([P, TCH, E]),
                op0=ALU.mult, op1=ALU.add)
            candU = sa1.tile([P, TCH, E], f32, tag="candU")
            nc.vector.tensor_tensor(out=candU[:], in0=base[:], in1=M[:],
                                    op=ALU.mult)
            nc.vector.tensor_scalar_add(candU[:], candU[:], -1.0)
            # pre-wrapped layout: region[p16*512 + (t//16)*8 + e] =
            # candU[t, e] so the expert-side read is a contiguous
            # [16, 512] tile (cols interleave experts, e innermost)
            for gg_ in range(8):
                nc.scalar.dma_start(
                    flatA[OFF_U:OFF_U + E * ntok]
                    .rearrange("(p t g e) -> g p t e", p=16, t=TCH,
                               g=8)[gg_],
                    candU[gg_ * 16:(gg_ + 1) * 16, :, :])

            # gate rows 2t+r of the payload (16 f32 = 64 B each)
            for r in range(2):
                g64 = sa.tile([P, TCH, GR], f32, tag="g64")
                nc.vector.tensor_scalar_add(
                    g64[:], g[r][:].to_broadcast([P, TCH, GR]), 0.0)
                nc.scalar.dma_start(
                    flatA[0:AGG * GR].rearrange(
                        "(tch p two f) -> p tch two f",
                        p=P, two=2, f=GR)[:, :, r, :],
                    g64[:])

            # b2 correction sum_r g_r * b2[e_r] -> ycsta (DRAM staging)
            wtok = sa1.tile([P, TCH, E], f32, tag="wtok")
            nc.vector.tensor_tensor(
                out=wtok[:], in0=A[0][:],
                in1=g[0][:].to_broadcast([P, TCH, E]), op=ALU.mult)
            wtk1 = sa.tile([P, TCH, E], f32, tag="wtk1")
            nc.vector.tensor_tensor(
                out=wtk1[:], in0=A[1][:],
                in1=g[1][:].to_broadcast([P, TCH, E]), op=ALU.mult)
            nc.vector.tensor_tensor(out=wtok[:], in0=wtok[:], in1=wtk1[:],
                                    op=ALU.add)
            wTe = sa1.tile([E, TCH * P], f32, tag="big8")
            for t in range(TCH):
                pw = pms.tile([E, P], f32, space="PSUM", tag="misc")
                nc.tensor.transpose(pw[:], wtok[:, t, :], id128sb[:])
                nc.vector.tensor_copy(wTe[:, t * P:(t + 1) * P], pw[:])
            for t in range(TCH):
                yc = ycp.tile([P, C], bf16, tag="yc")
                for hh in range(2):
                    pc = pms.tile([P, 512], f32, space="PSUM", tag="misc")
                    nc.tensor.matmul(pc[:], lhsT=wTe[:, t * P:(t + 1) * P],
                                     rhs=b2sb[:, hh * 512:(hh + 1) * 512],
                                     start=True, stop=True)
                    nc.vector.tensor_copy(yc[:, hh * 512:(hh + 1) * 512],
                                          pc[:])
                nc.scalar.dma_start(
                    ycsta[par, t * P:(t + 1) * P, :], yc[:])
            return dict(cnt16=cnt16)

        def do_ag(rep):
            par = rep % 2
            nc.gpsimd.collective_compute(
                "AllGather", ALU.bypass, replica_groups=rg,
                ins=[agin[par]],
                outs=[agout[par].rearrange("n w -> (n w)")])

        def do_a2a(rep):
            par = rep % 2
            nc.gpsimd.collective_compute(
                "AllToAll", ALU.bypass, replica_groups=rg,
                ins=[a2in[par, 0:NCORE * CAP]
                     .rearrange("(n s) c -> n s c", n=NCORE)],
                outs=[a2out[par]])

        def stage_e_pre(rep):
            """Expert-side index build: own-expert global slot list,
            A2A row offsets, gather/scatter index triple."""
            par = rep % 2
            # zero the A2A input (scatter_add needs a clean base)
            a2flat = a2in[par].rearrange("s c -> (s c)")
            zchunk = P * 1024
            nz = NCORE * CAP * C // zchunk
            for z in range(nz):
                nc.sync.dma_start(
                    a2flat[z * zchunk:(z + 1) * zchunk]
                    .rearrange("(p f) -> p f", p=P),
                    zt[:])

            flatO = agout[par].rearrange("n w -> (n w)")
            ctU = mb.tile([16, NCORE * 512], f32, tag="ctU")
            cpart = mb.tile([16, NCORE], f32, tag="cpart")
            for j in range(NCORE):
                rU = mb.tile([16, 512], f32, tag="rU")
                nc.sync.dma_start(
                    rU[:], flatO[j * NWORDS + OFF_U:
                                 j * NWORDS + OFF_U + E * ntok]
                    .rearrange("(p f) -> p f", p=16))
                # mask to own expert (others/pads -> -1), then rebase the
                # valid entries only: + j*ntok - 1 (u encodes t_local+1)
                uj = ctU[:, j * 512:(j + 1) * 512]
                nc.vector.scalar_tensor_tensor(
                    out=uj, in0=rU[:], scalar=1.0, in1=emsb[:],
                    op0=ALU.add, op1=ALU.mult)
                nc.vector.tensor_scalar_add(uj, uj, -1.0)
                vm = mb.tile([16, 512], f32, tag="vm")
                nc.vector.tensor_scalar(vm[:], uj, 0.0, 0.0,
                                        op0=ALU.is_ge, op1=ALU.add)
                nc.vector.scalar_tensor_tensor(
                    out=uj, in0=vm[:], scalar=float(j * ntok - 1),
                    in1=uj, op0=ALU.mult, op1=ALU.add)
                nc.vector.tensor_reduce(cpart[:, j:j + 1], vm[:],
                                        axis=AX.X, op=ALU.add)

            # per-home counts -> A2A row offsets off[j] = j*CAP - start_j
            chp = pms.tile([16, NCORE], f32, space="PSUM", tag="misc")
            nc.tensor.matmul(chp[:], lhsT=o16s[:], rhs=cpart[:],
                             start=True, stop=True)
            ch = mb.tile([16, NCORE], f32, tag="ch")
            nc.vector.tensor_copy(ch[:], chp[:])

            # two-level compaction (sparse_gather input free dim is capped
            # at 512 on HW): per-home 512 -> CAP16 (junk tail -> -1 via the
            # home count), then one gather over the padded concat.
            r2in = mb.tile([16, NCORE * CAP16], f32, tag="r2in")
            for j in range(NCORE):
                r1 = mb.tile([16, CAP16], f32, tag="r1")
                nc.vector.memset(r1[:], -1.0)
                nfd1 = mb.tile([1, 1], dt.uint32, tag="nfd1")
                nc.gpsimd.sparse_gather(
                    r1[:], ctU[:, j * 512:(j + 1) * 512],
                    num_found=nfd1[:])
                mj = mb.tile([16, CAP16], f32, tag="mj")
                nc.vector.tensor_tensor(
                    out=mj[:], in0=slotio[:, :CAP16],
                    in1=ch[:, j:j + 1].to_broadcast([16, CAP16]),
                    op=ALU.is_lt)
                nc.vector.tensor_scalar_add(r1[:], r1[:], 1.0)
                nc.vector.tensor_tensor(out=r1[:], in0=r1[:], in1=mj[:],
                                        op=ALU.mult)
                nc.vector.tensor_scalar_add(
                    r2in[:, j * CAP16:(j + 1) * CAP16], r1[:], -1.0)
            tkU = mb.tile([16, S16], f32, tag="tkU")
            nc.vector.memset(tkU[:], -1.0)
            nfd = mb.tile([1, 1], dt.uint32, tag="nfd")
            nc.gpsimd.sparse_gather(tkU[:], r2in[:], num_found=nfd[:])
            pf = ch
            for sh in (1, 2, 4):
                nxtp = mb.tile([16, NCORE], f32, tag=f"pf{sh}")
                nc.vector.tensor_copy(nxtp[:, 0:sh], pf[:, 0:sh])
                nc.vector.tensor_tensor(out=nxtp[:, sh:NCORE],
                                        in0=pf[:, sh:NCORE],
                                        in1=pf[:, 0:NCORE - sh], op=ALU.add)
                pf = nxtp
            excl = mb.tile([16, NCORE], f32, tag="excl")
            nc.vector.tensor_tensor(out=excl[:], in0=pf[:], in1=ch[:],
                                    op=ALU.subtract)
            off = mb.tile([16, NCORE], f32, tag="off")
            nc.vector.scalar_tensor_tensor(
                out=off[:], in0=io8[:], scalar=float(CAP), in1=excl[:],
                op0=ALU.mult, op1=ALU.subtract)
            total = pf[:, NCORE - 1:NCORE]

            # decode: rank bit, global token id (clamped), home, gate row
            rb = mb.tile([16, S16], f32, tag="rb")
            nc.vector.tensor_scalar(rb[:], tkU[:], RBIT, 0.0,
                                    op0=ALU.is_ge, op1=ALU.add)
            xf = mb.tile([16, S16], f32, tag="xf")
            nc.vector.scalar_tensor_tensor(
                out=xf[:], in0=rb[:], scalar=-RBIT, in1=tkU[:],
                op0=ALU.mult, op1=ALU.add)
            nc.vector.tensor_scalar(xf[:], xf[:], 0.0, float(NTOT - 1),
                                    op0=ALU.max, op1=ALU.min)
            js = mb.tile([16, S16], f32, tag="js")
            nc.vector.tensor_scalar(js[:], xf[:], float(ntok), 0.0,
                                    op0=ALU.is_ge, op1=ALU.add)
            for k in range(2, NCORE):
                jtmp = mb.tile([16, S16], f32, tag="jtmp")
                nc.vector.tensor_scalar(jtmp[:], xf[:], float(k * ntok),
                                        0.0, op0=ALU.is_ge, op1=ALU.add)
                nc.vector.tensor_tensor(out=js[:], in0=js[:], in1=jtmp[:],
                                        op=ALU.add)
            # gate row = 2*t_g + r + (AGR-2048)*home
            grow = mb.tile([16, S16], f32, tag="grow")
            nc.vector.scalar_tensor_tensor(
                out=grow[:], in0=xf[:], scalar=2.0, in1=rb[:],
                op0=ALU.mult, op1=ALU.add)
            gtmp = mb.tile([16, S16], f32, tag="gtmp")
            nc.vector.scalar_tensor_tensor(
                out=gtmp[:], in0=js[:], scalar=float(GROWS - AGG),
                in1=grow[:], op0=ALU.mult, op1=ALU.add)
            # scatter row = slot + off[home]; pads (slot >= total) -> -1
            osel = mb.tile([16, S16], f32, tag="osel")
            nc.vector.memset(osel[:], 0.0)
            for k in range(NCORE):
                ieq = mb.tile([16, S16], f32, tag="ieq")
                nc.vector.tensor_scalar(ieq[:], js[:], float(k), 0.0,
                                        op0=ALU.is_equal, op1=ALU.add)
                nc.vector.tensor_tensor(
                    out=ieq[:], in0=ieq[:],
                    in1=off[:, k:k + 1].to_broadcast([16, S16]),
                    op=ALU.mult)
                nc.vector.tensor_tensor(out=osel[:], in0=osel[:],
                                        in1=ieq[:], op=ALU.add)
            scat = mb.tile([16, S16], f32, tag="scat")
            nc.vector.tensor_tensor(out=scat[:], in0=slotio[:, :S16],
                                    in1=osel[:], op=ALU.add)
            # junk tail -> trash row NCORE*CAP (payload junk collides only
            # in trash; skipped/negative indices hang the SWDGE scatter)
            msk = mb.tile([16, S16], f32, tag="msk")
            nc.vector.tensor_tensor(out=msk[:], in0=slotio[:, :S16],
                                    in1=total.to_broadcast([16, S16]),
                                    op=ALU.is_lt)
            nc.vector.tensor_scalar_add(scat[:], scat[:],
                                        float(-NCORE * CAP))
            nc.vector.tensor_tensor(out=scat[:], in0=scat[:], in1=msk[:],
                                    op=ALU.mult)
            nc.vector.tensor_scalar_add(scat[:], scat[:],
                                        float(NCORE * CAP))

            if debug and rep == 0:
                nc.scalar.dma_start(dbgA[0], tkU[:])
                nc.scalar.dma_start(dbgA[1], xf[:])
                nc.scalar.dma_start(dbgA[2], scat[:])
                nc.scalar.dma_start(dbgA[3], gtmp[:])
                nc.scalar.dma_start(dbgC[0], ch[:])
                nc.scalar.dma_start(dbgC[1], off[:])
                nc.scalar.dma_start(dbgC[2], cpart[:])
            trip = tpl.tile([P, 3 * S16], dt.int16, tag="trip")
            nc.vector.tensor_copy(trip[0:16, 0:S16], xf[:])
            nc.vector.tensor_copy(trip[0:16, S16:2 * S16], scat[:])
            nc.vector.tensor_copy(trip[0:16, 2 * S16:3 * S16], gtmp[:])
            for sz in (16, 32, 64):
                nc.scalar.dma_start(trip[sz:2 * sz, :], trip[0:sz, :])
            return trip

        def stage_e_main(rep, trip):
            """Expert MLP from resident weights; scatter to A2A input."""
            par = rep % 2
            t16X = trip[:, 0:S16]
            t16S = trip[:, S16:2 * S16]
            t16G = trip[:, 2 * S16:3 * S16]
            for (hoff, W) in HALVES:
                h16 = hoff // 16
                ggt = ggp.tile([P, W // 128, GW], f32, tag="gg")
                nc.gpsimd.dma_gather(
                    ggt[:], agout[par].rearrange("n w -> (n w)")
                    .rearrange("(r g) -> r g", g=GR),
                    t16G[:, h16:h16 + W // 16], W, W, GW)
                if level < 4:
                    continue
                xg = xgp.tile([P, CK, W], bf16, tag="xg")
                nc.gpsimd.dma_gather(xg[:], xtm,
                                     t16X[:, h16:h16 + W // 16],
                                     W, W, C, transpose=True)
                if level < 5:
                    continue
                hs = []
                for hk in range(HK):
                    ps = p1.tile([P, W], f32, space="PSUM", tag="ps1")
                    for k in range(CK):
                        nc.tensor.matmul(
                            ps[:], lhsT=w1t[hk][:, k * P:(k + 1) * P],
                            rhs=xg[:, k, :],
                            start=(k == 0), stop=(k == CK - 1))
                    ht = hp.tile([P, W], bf16, tag=f"h{hk}")
                    nc.scalar.activation(ht[:], ps[:], AF.Relu,
                                         bias=b1sb[:, hk:hk + 1])
                    hs.append(ht)
                if level < 6:
                    continue
                yst = ystp.tile([P, W // 128, C], bf16, tag="yst")
                for sub in range(W // 128):
                    lo = sub * 128
                    psA = p2.tile([P, 512], f32, space="PSUM",
                                  tag="ps2a")
                    psB = p2.tile([P, 512], f32, space="PSUM",
                                  tag="ps2b")
                    for hk in range(HK):
                        nc.tensor.matmul(
                            psA[:], lhsT=hs[hk][:, lo:lo + 128],
                            rhs=w2t[hk][:, 0:512],
                            start=(hk == 0), stop=(hk == HK - 1))
                        nc.tensor.matmul(
                            psB[:], lhsT=hs[hk][:, lo:lo + 128],
                            rhs=w2t[hk][:, 512:1024],
                            start=(hk == 0), stop=(hk == HK - 1))
                    nc.vector.tensor_tensor(
                        out=yst[:, sub, 0:512], in0=psA[:],
                        in1=ggt[:, sub, 0:1].to_broadcast([P, 512]),
                        op=ALU.mult)
                    nc.vector.tensor_tensor(
                        out=yst[:, sub, 512:1024], in0=psB[:],
                        in1=ggt[:, sub, 0:1].to_broadcast([P, 512]),
                        op=ALU.mult)
                if level < 7:
                    continue
                nc.gpsimd.dma_scatter_add(
                    a2in[par], yst[:], t16S[:, h16:h16 + W // 16],
                    W, W, C, queue_num=1)

        def stage_c_pre(rep, actx):
            """Home-side combine lists (own tokens per expert)."""
            par = rep % 2
            cnt16 = actx["cnt16"]
            t16s = []
            for e2 in range(NCORE):
                cw = cbp.tile([16, GW], f32, tag="cw")
                nc.scalar.dma_start(
                    cw[:], agin[par, OFF_U:OFF_U + E * ntok]
                    .rearrange("(p f e) -> p f e", p=16, e=E)[:, :, e2])
                msk = cbp.tile([16, CAP16], f32, tag="cmsk")
                nc.vector.tensor_tensor(
                    out=msk[:], in0=slotio[:, :CAP16],
                    in1=cnt16[:, e2:e2 + 1].to_broadcast([16, CAP16]),
                    op=ALU.is_lt)
                tk = cbp.tile([16, CAP16], f32, tag="ctk")
                nc.vector.memset(tk[:], -1.0)
                nfd = cbp.tile([1, 1], dt.uint32, tag="cnfd")
                nc.gpsimd.sparse_gather(tk[:, :CAP16], cw[:],
                                        num_found=nfd[:])
                # decode t+1 out of candU = t + 1 + RBIT*r
                rb = cbp.tile([16, CAP16], f32, tag="crb")
                nc.vector.tensor_scalar(rb[:], tk[:], RBIT, 0.0,
                                        op0=ALU.is_ge, op1=ALU.add)
                xff = cbp.tile([16, CAP16], f32, tag="cxff")
                nc.vector.scalar_tensor_tensor(
                    out=xff[:], in0=rb[:], scalar=-RBIT, in1=tk[:],
                    op0=ALU.mult, op1=ALU.add)
                nc.vector.tensor_scalar(xff[:], xff[:], 1.0, float(ntok),
                                        op0=ALU.max, op1=ALU.min)
                nc.vector.tensor_scalar_add(xff[:], xff[:], -1.0)
                stf = cbp.tile([16, CAP16], f32, tag="cstf")
                nc.vector.scalar_tensor_tensor(
                    out=stf[:], in0=xff[:], scalar=float(-ntok),
                    in1=msk[:], op0=ALU.add, op1=ALU.mult)
                nc.vector.tensor_scalar_add(stf[:], stf[:], float(ntok))
                if debug and rep == 0:
                    nc.scalar.dma_start(dbgH[e2], stf[:])
                t16 = clp.tile([P, CAP16], dt.int16, tag=f"ct16_{e2}")
                nc.vector.tensor_copy(t16[0:16, :], stf[:])
                for sz in (16, 32, 64):
                    nc.scalar.dma_start(t16[sz:2 * sz, :], t16[0:sz, :])
                t16s.append(t16)
            return t16s

        def stage_c_main(rep, t16s):
            """yT = b2 correction + scatter-add of returned rows."""
            par = rep % 2
            nc.sync.dma_start(yT[0:ntok, :], ycsta[par])
            for e2 in range(NCORE):
                bt = cbp.tile([P, CAP // 128, C], bf16, tag="bt")
                nc.sync.dma_start(
                    bt[:], a2out[par, e2].rearrange("(s p) c -> p s c",
                                                    p=P))
                nc.gpsimd.dma_scatter_add(yT, bt[:], t16s[e2][:], CAP, CAP,
                                          C, queue_num=1)

        # software pipeline: E_pre(r)/C_pre(r) first (need only AG(r)/
        # A(r)), then A(r+1)+AG(r+1), then the expert MLP of r
        actxs = {0: stage_a(0)}
        if level >= 1:
            do_ag(0)
        for rep in range(repeat):
            trip = stage_e_pre(rep) if level >= 2 else None
            cl = (stage_c_pre(rep, actxs.pop(rep))
                  if level >= 9 else None)
            if rep + 1 < repeat:
                actxs[rep + 1] = stage_a(rep + 1)
                if level >= 1:
                    do_ag(rep + 1)
            if level >= 3:
                stage_e_main(rep, trip)
            if level >= 8:
                do_a2a(rep)
            if level >= 9:
                stage_c_main(rep, cl)

    return nc


# ---------------- host side ----------------

def _host_route(xf, gate_w, gate_b):
    logits = xf.astype(np.float32) @ gate_w.astype(np.float32) + gate_b
    return np.argpartition(-logits, TOPK - 1, axis=1)[:, :TOPK]


def _host_caps(order, ntok=NTOK):
    cnt = np.bincount(order.ravel(), minlength=E)
    slot = int(np.ceil((cnt.max() + 16) / 128.0) * 128)
    ncore = order.shape[0] // ntok
    pair = np.zeros((E, ncore), np.int64)
    for j in range(ncore):
        sl = order[j * ntok:(j + 1) * ntok]
        pair[:, j] = np.bincount(sl.ravel(), minlength=E)
    cap = int(np.ceil((pair.max() + 16) / 128.0) * 128)
    assert slot // 16 <= 512, f"slot overflow: {slot}"
    return slot, cap


def kernel(x, gate_w, gate_b, w1, b1, w2, b2):
    from concourse.bass_utils import run_bass_kernel_spmd
    import ml_dtypes

    x = np.asarray(x, np.float32)
    gate_w = np.asarray(gate_w, np.float32)
    gate_b = np.asarray(gate_b, np.float32)
    w1 = np.asarray(w1, np.float32)
    b1 = np.asarray(b1, np.float32)
    w2 = np.asarray(w2, np.float32)
    b2 = np.asarray(b2, np.float32)

    # w1 in lhsT-chunk layout: [E, HK, P(c in chunk), CK*P(h)]
    w1r = np.ascontiguousarray(
        (w1.reshape(E, CK, P, HK, P).transpose(0, 3, 2, 1, 4)
         .reshape(E, HK, P, C)).astype(ml_dtypes.bfloat16))
    w2b = np.ascontiguousarray(
        w2.reshape(E, HK, P, C).astype(ml_dtypes.bfloat16))

    b, t, c = x.shape
    xf = x.reshape(b * t, c)
    order = _host_route(xf, gate_w, gate_b)
    S = _host_caps(order)
    nc = build_program(S)

    xtm_full = np.ascontiguousarray(xf.astype(ml_dtypes.bfloat16))
    shared = {
        "xtm": xtm_full,
        "gw": gate_w,
        "gb": gate_b.reshape(E, 1).copy(),
        "b2e": b2,
        "id8": np.eye(E, dtype=np.float32),
        "id128": np.eye(P, dtype=np.float32),
    }
    in_maps = []
    for cc in range(NCORE):
        sl = xf[cc * NTOK:(cc + 1) * NTOK]
        m = dict(shared)
        m["xT"] = np.ascontiguousarray(sl.T)
        m["w1"] = w1r[cc]
        m["b1"] = np.ascontiguousarray(b1[cc].reshape(H, 1))
        m["w2"] = w2b[cc]
        em = np.zeros((16, 64, E), np.float32)
        em[:, :, cc] = 1.0
        m["emask"] = em.reshape(16, 512)
        in_maps.append(m)

    global LAST_BUILD, LAST_S
    LAST_BUILD = (nc, in_maps)
    LAST_S = S
    res = run_bass_kernel_spmd(nc, in_maps, core_ids=list(range(NCORE)))
    outs = [np.asarray(r["yT"][:NTOK]).astype(np.float32)
            for r in res.results]
    y = np.concatenate(outs, axis=0).reshape(b, t, c)
    return y
